# revision 40
# baseline (speedup 1.0000x reference)
"""Trainium2 Bass kernel for single-head causal attention with dropout.

reference:
    q,k,v = x@Wq, x@Wk, x@Wv          [B,T,H]
    wei = softmax(mask(q@k^T * H**-0.5))   (causal)
    wei = wei * (drop_u >= 0.2)/0.8
    out = wei @ v                      [B,T,H]

B=16, T=2048, D=1024, H=64. 8 NeuronCores, data-parallel over batch
(2 batches/core).

Design notes (v3):
- Everything on-chip is fp16 (matmuls 1 cyc/row at any size, half the
  HBM bytes for x). Accumulations in f32 PSUM.
- The dropout mask ships from host as an exact fp16 keep-mask
  {0, 1.25}; dropout is a plain DVE tensor_tensor multiply.
- Scores transposed S^T[s, q] in [128 x 512] chunks; full chunks
  processed in pairs sharing one 2-bank PSUM tile so a single exp
  covers 1024 columns. Causal mask via a bf-style -30000 add-matmul
  on the diagonal blocks (exp underflows to exact 0).
- Softmax denominator: per-chunk ones^T @ E matmuls accumulated in a
  [1,512] PSUM bank (PE cost is tiny vs DVE alternatives measured on
  HW); 1/d via ScalarE exp(-ln d) (DVE single-partition reciprocal
  measured 4us(!) per call).
- Output stores via gpsimd SWDGE with fp16->f32 cast on the fly.
- Group epilogues (1/d, normalize, transpose-out) are DEFERRED into
  the next group's pipeline so the tensor engine never sits in the
  ln->exp->rdbc dependency bubble; ot and dps share one 2-buffered
  PSUM bank to make that legal within 8 banks.
"""

import numpy as np
from contextlib import ExitStack


def _ensure_ntff_hook():
    """The agent image's `antenv` lacks `axon_hooks`, so trn_boot's NTFF
    profile hook registration degrades silently and trace=True dies on
    import. Provide the missing module + register the ctypes hook."""
    import sys, types
    try:
        from antenv.axon_hooks import get_axon_ntff_profile_hook  # noqa
        return  # real module present
    except ImportError:
        pass
    try:
        import antenv
        mod = types.ModuleType("antenv.axon_hooks")
        _holder = [None]
        mod.set_axon_ntff_profile_hook = lambda h: _holder.__setitem__(0, h)
        mod.get_axon_ntff_profile_hook = lambda: _holder[0]
        sys.modules["antenv.axon_hooks"] = mod
        antenv.axon_hooks = mod
        from trn_agent_boot.trn_boot import _ntff_profile_via_ctypes
        mod.set_axon_ntff_profile_hook(
            _ntff_profile_via_ctypes("/opt/axon/libaxon_pjrt.so"))
    except Exception:
        pass


_ensure_ntff_hook()

import concourse.bass as bass
import concourse.tile as tile
from concourse import mybir
from concourse.bass_utils import run_bass_kernel_spmd
from concourse.masks import make_identity

F32 = mybir.dt.float32
F32R = mybir.dt.float32r
F16 = mybir.dt.float16

B, T, D, H = 16, 2048, 1024, 64
N_CORES = 8
BPC = B // N_CORES          # batches per core
P_DROP = 0.2
NB = T // 128               # 16 key chunks per batch
NG = T // 512               # 4 query groups per batch
GROUP = 4                   # key chunks per query group
MASK_NEG = -30000.0         # causal mask addend (fp16-representable)


# walrus here allows only ONE sync-wait command per instruction; Tile can
# attach several (e.g. its exit drain). Move extras onto same-engine NOPs.
def _split_excess_waits(nc):
    n = 0
    for f in nc.m.functions:
        for bb in f.blocks:
            new_insts = []
            changed = False
            for inst in bb.instructions:
                si = inst.sync_info
                if si is not None and si.on_wait and len(si.on_wait) > 1:
                    waits = list(si.on_wait)
                    extra, keep = waits[:-1], waits[-1:]
                    for i, w in enumerate(extra):
                        new_insts.append(mybir.InstNoOp(
                            name=f"{inst.name}-ws-{i}",
                            engine=inst.engine, ins=[], outs=[],
                            sync_info=mybir.SyncInfo(on_wait=[w], on_update=[]),
                            text_hint="waitsplit", bass_nofuse=True))
                        n += 1
                    si.on_wait = keep
                    changed = True
                new_insts.append(inst)
            if changed:
                bb.instructions[:] = new_insts
    return n


def _build(ctx: ExitStack, tc: "tile.TileContext", xt, wqk, wv, kt, out):
    nc = tc.nc
    AF = mybir.ActivationFunctionType
    OP = mybir.AluOpType

    cpool = ctx.enter_context(tc.tile_pool(name="const", bufs=1))
    xpool = ctx.enter_context(tc.tile_pool(name="xt", bufs=2))
    qkvpool = ctx.enter_context(tc.tile_pool(name="qkv", bufs=2))
    vtpool = ctx.enter_context(tc.tile_pool(name="vt", bufs=2))
    kfpool = ctx.enter_context(tc.tile_pool(name="kf", bufs=2))
    kdpool = ctx.enter_context(tc.tile_pool(name="kd", bufs=4))
    eppool = ctx.enter_context(tc.tile_pool(name="ep", bufs=3))
    edpool = ctx.enter_context(tc.tile_pool(name="ed", bufs=3))
    pppool = ctx.enter_context(tc.tile_pool(name="pp", bufs=3))
    pdpool = ctx.enter_context(tc.tile_pool(name="pd", bufs=3))
    otsbpool = ctx.enter_context(tc.tile_pool(name="otsb", bufs=2))
    onsbpool = ctx.enter_context(tc.tile_pool(name="onsb", bufs=2))
    outpool = ctx.enter_context(tc.tile_pool(name="outsb", bufs=4))
    rdpool = ctx.enter_context(tc.tile_pool(name="rd", bufs=2))

    # PSUM: Sp 2x[128,1024] = 4 banks (pairs, v-proj, v-stage),
    # Sd 3x[128,512] = 3 banks (qk-proj, diag scores, rdbc),
    # combo (ot rows 0:64 + denom row 64) 1x[128,512] = 1 bank -> 8 total.
    pspool = ctx.enter_context(tc.tile_pool(name="psp", bufs=2, space="PSUM"))
    psdool = ctx.enter_context(tc.tile_pool(name="psd", bufs=3, space="PSUM"))
    combops = ctx.enter_context(tc.tile_pool(name="combo", bufs=1, space="PSUM"))

    # deferred group epilogues: each entry is a closure that emits the
    # PE/DVE tail of a finished group; flushed between the next group's
    # producers so the tensor engine never idles in the 1/d dependency
    # chain.
    eptail = []

    def flush_eptail():
        while eptail:
            eptail.pop(0)()

    # ---- constants -------------------------------------------------------
    ident_h = cpool.tile([128, 128], F16)
    make_identity(nc, ident_h[:])

    ones_h = cpool.tile([128, 1], F16)
    nc.gpsimd.memset(ones_h[:], 1.0)
    ones64_h = cpool.tile([1, 64], F16)
    nc.gpsimd.memset(ones64_h[:], 1.0)

    wqk_sb = cpool.tile([128, 8 * 128], F16)
    nc.sync.dma_start(
        wqk_sb[:].rearrange("p (c h) -> p c h", c=8),
        wqk.rearrange("(c p) h -> p c h", p=128))
    wv_sb = cpool.tile([128, 8 * H], F16)
    nc.sync.dma_start(
        wv_sb[:].rearrange("p (c h) -> p c h", c=8),
        wv.rearrange("(c p) h -> p c h", p=128))

    # staged x prefetch: two quarters of lead, issued on the Pool SWDGE
    # ring so they never pile up in front of the SP-ring keep-mask loads
    xbigs = {}

    def load_x(b, quarter):
        if b >= BPC or quarter >= 4 or (b, quarter) in xbigs:
            return
        xb = xpool.tile([128, 8 * 512], F16, tag=f"x{b}{quarter}", bufs=1)
        nc.gpsimd.dma_start(
            xb[:].rearrange("p (c t) -> p c t", c=8),
            xt[b].rearrange("(c p) t -> p c t", p=128)
              [:, :, 512 * quarter:512 * (quarter + 1)])
        xbigs[(b, quarter)] = xb

    # diag keep-mask prefetch, one group of lead
    kds = {}

    def load_kd(b, g):
        if b >= BPC or g >= NG or (b, g) in kds:
            return
        tiles = []
        for t in range(4 * g, 4 * g + 4):
            qo = 128 * (t - 4 * g)
            kd = kdpool.tile([128, 512], F16, tag="kd", bufs=8)
            nc.sync.dma_start(
                kd[:, qo:512],
                kt[b, 128 * t:128 * (t + 1), 512 * g + qo:512 * (g + 1)])
            tiles.append(kd)
        kds[(b, g)] = tiles

    load_x(0, 0)
    load_x(0, 1)
    load_kd(0, 0)

    for b in range(BPC):
        # ---- phase A: projections ---------------------------------------
        # qkT[0:64,:] = q^T, qkT[64:128,:] = k^T ; v natural [s, H] tiles
        qkT = qkvpool.tile([128, T], F16, tag="qkT")
        kT0 = qkvpool.tile([64, T], F16, tag="kT0")
        vT = vtpool.tile([64, T], F16, tag="vT")
        v_sb = qkvpool.tile([128, NB * H], F16, tag="v")

        for quarter in range(4):
            if quarter + 2 < 4:
                load_x(b, quarter + 2)
            else:
                load_x(b + 1, quarter + 2 - 4)
            col = 512 * quarter
            xb = xbigs.pop((b, quarter))
            ps = psdool.tile([128, 512], F32, tag="Sd")
            for c in range(8):
                nc.tensor.matmul(
                    ps[:], wqk_sb[:, 128 * c:128 * (c + 1)],
                    xb[:, 512 * c:512 * (c + 1)],
                    start=(c == 0), stop=(c == 7))
            nc.vector.tensor_copy(qkT[:, col:col + 512], ps[:])
            # matmul needs lhsT/rhs at the same base partition: move k^T
            # (psum rows 64..127) down to partitions 0..63 via DMA
            nc.sync.dma_start(kT0[:, col:col + 512], qkT[64:128, col:col + 512])
            ps2 = pspool.tile([64, 512], F32, tag="Sp",
                              padded_shape=[128, 1024])
            for c in range(8):
                nc.tensor.matmul(
                    ps2[:], wv_sb[:, H * c:H * (c + 1)],
                    xb[:, 512 * c:512 * (c + 1)],
                    start=(c == 0), stop=(c == 7))
            nc.scalar.copy(vT[:, col:col + 512], ps2[:])
            if quarter == 0:
                flush_eptail()  # prev batch's last-group tail
        qT = qkT
        kT = kT0

        # v: [64,T] -> natural [s, H] tiles, 8 PE transposes per PSUM bank
        for m in range(2):
            stage = pspool.tile([128, 512], F16, tag="Sp",
                                padded_shape=[128, 2048])
            for tloc in range(8):
                t = 8 * m + tloc
                nc.tensor.transpose(
                    stage[:, H * tloc:H * (tloc + 1)],
                    vT[:, 128 * t:128 * (t + 1)], ident_h[:64, :64])
            nc.vector.tensor_copy(
                v_sb[:, H * 8 * m:H * 8 * (m + 1)], stage[:])

        # ---- phase B: attention, per query group of 512 ------------------
        kfs = {}

        def load_kf(g):
            if g > 3 or 4 * g == 0:
                return
            nf = 4 * g
            kf = kfpool.tile([128, nf * 512], F16, tag="kf",
                             padded_shape=[128, 12 * 512])
            nc.sync.dma_start(
                kf[:, :nf * 512].rearrange("p (c q) -> p c q", c=nf),
                kt[b, 0:128 * nf, 512 * g:512 * (g + 1)]
                  .rearrange("(c p) q -> p c q", p=128))
            kfs[g] = kf

        load_kf(1)
        for g in range(NG):
            if g >= 1:
                load_kf(g + 1)
            if g + 1 < NG:
                load_kd(b, g + 1)
            else:
                load_kd(b + 1, 0)
            qcol = 512 * g
            kd_tiles = kds.pop((b, g))
            kf = kfs.pop(g, None)
            combo = combops.tile([128, 512], F32, tag="combo")
            ot = combo[0:64, :]
            dps = combo[64:65, :]

            # work items: pairs of full chunks, then the 4 diagonal chunks
            items = [("pair", 2 * i) for i in range(2 * g)]
            items += [("diag", t) for t in range(4 * g, 4 * g + 4)]
            n_items = len(items)
            prod = {}

            def produce(i):
                kind, t = items[i]
                if kind == "pair":
                    sp = pspool.tile([128, 1024], F32, tag="Sp")
                    nc.tensor.matmul(
                        sp[:, 0:512], kT[:, 128 * t:128 * (t + 1)],
                        qT[0:64, qcol:qcol + 512], start=True, stop=True)
                    nc.tensor.matmul(
                        sp[:, 512:1024], kT[:, 128 * (t + 1):128 * (t + 2)],
                        qT[0:64, qcol:qcol + 512], start=True, stop=True)
                    E = eppool.tile([128, 1024], F16, tag="Ep")
                    nc.scalar.activation(
                        E[:], sp[:], AF.Exp, scale=float(H) ** -0.5)
                    prod[i] = E
                else:
                    qo = 128 * (t - 4 * g)
                    kd = kd_tiles[t - 4 * g]
                    sd = psdool.tile([128, 512], F32, tag="Sd")
                    nc.tensor.matmul(
                        sd[:, qo:512], kT[:, 128 * t:128 * (t + 1)],
                        qT[0:64, qcol + qo:qcol + 512],
                        start=True, stop=True)
                    E = edpool.tile([128, 512], F16, tag="Ed")
                    nc.scalar.activation(
                        E[:, qo:512], sd[:, qo:512], AF.Exp,
                        scale=float(H) ** -0.5)
                    # causal mask: zero E above the diagonal of the
                    # 128x128 diagonal block (Pool; keeps PE out of it)
                    nc.gpsimd.affine_select(
                        out=E[:, qo:qo + 128], in_=E[:, qo:qo + 128],
                        compare_op=OP.is_ge, fill=0.0,
                        base=0, pattern=[[1, 128]], channel_multiplier=-1)
                    prod[i] = (E, kd)

            def consume(i):
                kind, t = items[i]
                if kind == "pair":
                    E = prod.pop(i)
                    # denominator contributions (pre-dropout), written to
                    # partition row 64 of the shared ot/dps bank
                    nc.tensor.matmul(
                        combo[64:65, :], ones_h[:], E[:, 0:512],
                        start=(i == 0), stop=False, skip_group_check=True)
                    nc.tensor.matmul(
                        combo[64:65, :], ones_h[:], E[:, 512:1024],
                        start=False, stop=False, skip_group_check=True)
                    Pp = pppool.tile([128, 1024], F16, tag="Pp")
                    nc.vector.tensor_mul(
                        Pp[:], kf[:, 512 * t:512 * (t + 2)], E[:])
                    nc.tensor.matmul(
                        ot[:], v_sb[:, H * t:H * (t + 1)], Pp[:, 0:512],
                        start=(i == 0), stop=False)
                    nc.tensor.matmul(
                        ot[:], v_sb[:, H * (t + 1):H * (t + 2)],
                        Pp[:, 512:1024],
                        start=False, stop=False)
                else:
                    qo = 128 * (t - 4 * g)
                    E, kd = prod.pop(i)
                    nc.tensor.matmul(
                        combo[64:65, qo:512], ones_h[:], E[:, qo:512],
                        start=(i == 0), stop=(i == n_items - 1),
                        skip_group_check=True)
                    Pd = pdpool.tile([128, 512], F16, tag="Pd")
                    nc.vector.tensor_mul(
                        Pd[:, qo:512], kd[:, qo:512], E[:, qo:512])
                    nc.tensor.matmul(
                        ot[:, qo:512], v_sb[:, H * t:H * (t + 1)],
                        Pd[:, qo:512],
                        start=(i == 0), stop=(i == n_items - 1))

            # software-pipelined: consumers trail producers by PD items and
            # are emitted in pairs, giving the tensor engine longer
            # back-to-back matmul runs. The previous group's epilogue tail
            # is flushed between the first producers so its PE ops land
            # when their inputs are long since ready.
            PD = 2
            pend = []
            for i in range(n_items):
                produce(i)
                if i in (1, 2) and eptail:
                    eptail.pop(0)()
                if i >= PD:
                    pend.append(i - PD)
                    if len(pend) == 2:
                        consume(pend[0])
                        consume(pend[1])
                        pend = []
            for i in pend:
                consume(i)
            for i in range(max(0, n_items - PD), n_items):
                consume(i)

            # ---- group epilogue -----------------------------------------
            # immediate part: free dps/ot quickly. 1/d as exp(-ln d) on
            # ScalarE (a [1,512] DVE reciprocal measured ~4us on HW).
            ln_d = rdpool.tile([1, 512], F32, tag="rdf")
            nc.scalar.activation(ln_d[:], combo[64:65, :], AF.Ln)
            rd = rdpool.tile([1, 512], F16, tag="rd")
            nc.scalar.activation(rd[:], ln_d[:], AF.Exp, scale=-1.0)
            ot_sb = otsbpool.tile([64, 512], F32, tag="otsb")
            nc.vector.tensor_copy(ot_sb[:], combo[0:64, :])

            def make_tail(b, g, rd, ot_sb):
                def tail_pe():
                    rdbc = psdool.tile([64, 512], F32, tag="Sd")
                    nc.tensor.matmul(rdbc[:], ones64_h[:], rd[:],
                                     start=True, stop=True)
                    on_sb = onsbpool.tile([64, 512], F16, tag="onsb")
                    nc.vector.tensor_mul(on_sb[:], ot_sb[:], rdbc[:])
                    tail_pe.on_sb = on_sb

                def tail_out():
                    on_sb = tail_pe.on_sb
                    stage = psdool.tile([128, 256], F16, tag="Sd",
                                        padded_shape=[128, 1024])
                    for cc in range(GROUP):
                        nc.tensor.transpose(
                            stage[:, 64 * cc:64 * (cc + 1)],
                            on_sb[:, 128 * cc:128 * (cc + 1)],
                            ident_h[:64, :64])
                    osb = outpool.tile([128, 256], F16, tag="osb")
                    nc.vector.tensor_copy(osb[:], stage[:])
                    # store with fp16->f32 cast via SWDGE
                    nc.gpsimd.dma_start(
                        out[b].rearrange("(c p) h -> p c h", p=128)
                           [:, GROUP * g:GROUP * (g + 1), :],
                        osb[:].rearrange("p (c h) -> p c h", c=GROUP))
                return [tail_pe, tail_out]

            eptail.extend(make_tail(b, g, rd, ot_sb))

    flush_eptail()


_CACHE = {}


def _get_nc():
    if "nc" not in _CACHE:
        nc = bass.Bass("TRN2", target_bir_lowering=False)
        xt = nc.dram_tensor("xt", [BPC, D, T], F16, kind="ExternalInput")
        wqk = nc.dram_tensor("wqk", [D, 128], F16, kind="ExternalInput")
        wv = nc.dram_tensor("wv", [D, H], F16, kind="ExternalInput")
        kt = nc.dram_tensor("kt", [BPC, T, T], F16, kind="ExternalInput")
        out = nc.dram_tensor("out", [BPC, T, H], F32, kind="ExternalOutput")
        with tile.TileContext(nc) as tc:
            with ExitStack() as ctx:
                _build(ctx, tc, xt.ap(), wqk.ap(), wv.ap(), kt.ap(), out.ap())
        _split_excess_waits(nc)
        _CACHE["nc"] = nc
    return _CACHE["nc"]


def kernel(x, Wq, Wk, Wv, drop_u, _trace=False):
    x = np.asarray(x)
    drop_u = np.asarray(drop_u)

    nc = _get_nc()
    xt = np.ascontiguousarray(
        x.astype(np.float16).transpose(0, 2, 1))           # [B, D, T]
    keep = (np.asarray(drop_u) >= np.float32(P_DROP))
    keep = (keep.astype(np.float16) * np.float16(1.0 / (1.0 - P_DROP)))
    kt = np.ascontiguousarray(keep.transpose(0, 2, 1))     # [B, T_s, T_q]
    wqk = np.ascontiguousarray(
        np.concatenate([np.asarray(Wq), np.asarray(Wk)], axis=1)
        .astype(np.float16))                               # [D, 128]
    wv16 = np.asarray(Wv).astype(np.float16)
    in_maps = []
    for c in range(N_CORES):
        lo = BPC * c
        in_maps.append({
            "xt": xt[lo:lo + BPC],
            "wqk": wqk, "wv": wv16,
            "kt": kt[lo:lo + BPC],
        })
    res = run_bass_kernel_spmd(
        nc, in_maps, core_ids=list(range(N_CORES)), trace=_trace)
    out = np.concatenate([res.results[c]["out"] for c in range(N_CORES)], axis=0)
    if _trace:
        kernel.last_exec_time_ns = res.exec_time_ns
        kernel.last_results = res
    return out


# revision 50
# speedup vs baseline: 1.0057x; 1.0057x over previous
"""Trainium2 Bass kernel for single-head causal attention with dropout.

reference:
    q,k,v = x@Wq, x@Wk, x@Wv          [B,T,H]
    wei = softmax(mask(q@k^T * H**-0.5))   (causal)
    wei = wei * (drop_u >= 0.2)/0.8
    out = wei @ v                      [B,T,H]

B=16, T=2048, D=1024, H=64. 8 NeuronCores, data-parallel over batch
(2 batches/core).

Design notes (v3):
- Everything on-chip is fp16 (matmuls 1 cyc/row at any size, half the
  HBM bytes for x). Accumulations in f32 PSUM.
- The dropout mask ships from host as an exact fp16 keep-mask
  {0, 1.25}; dropout is a plain DVE tensor_tensor multiply.
- Scores transposed S^T[s, q] in [128 x 512] chunks; full chunks
  processed in pairs sharing one 2-bank PSUM tile so a single exp
  covers 1024 columns. Causal mask via a bf-style -30000 add-matmul
  on the diagonal blocks (exp underflows to exact 0).
- Softmax denominator: per-chunk ones^T @ E matmuls accumulated in a
  [1,512] PSUM bank (PE cost is tiny vs DVE alternatives measured on
  HW); 1/d via ScalarE exp(-ln d) (DVE single-partition reciprocal
  measured 4us(!) per call).
- Output stores via gpsimd SWDGE with fp16->f32 cast on the fly.
- Group epilogues (1/d, normalize, transpose-out) are DEFERRED into
  the next group's pipeline so the tensor engine never sits in the
  ln->exp->rdbc dependency bubble; ot and dps share one 2-buffered
  PSUM bank to make that legal within 8 banks.
"""

import numpy as np
from contextlib import ExitStack


def _ensure_ntff_hook():
    """The agent image's `antenv` lacks `axon_hooks`, so trn_boot's NTFF
    profile hook registration degrades silently and trace=True dies on
    import. Provide the missing module + register the ctypes hook."""
    import sys, types
    try:
        from antenv.axon_hooks import get_axon_ntff_profile_hook  # noqa
        return  # real module present
    except ImportError:
        pass
    try:
        import antenv
        mod = types.ModuleType("antenv.axon_hooks")
        _holder = [None]
        mod.set_axon_ntff_profile_hook = lambda h: _holder.__setitem__(0, h)
        mod.get_axon_ntff_profile_hook = lambda: _holder[0]
        sys.modules["antenv.axon_hooks"] = mod
        antenv.axon_hooks = mod
        from trn_agent_boot.trn_boot import _ntff_profile_via_ctypes
        mod.set_axon_ntff_profile_hook(
            _ntff_profile_via_ctypes("/opt/axon/libaxon_pjrt.so"))
    except Exception:
        pass


_ensure_ntff_hook()

import concourse.bass as bass
import concourse.tile as tile
from concourse import mybir
from concourse.bass_utils import run_bass_kernel_spmd
from concourse.masks import make_identity

F32 = mybir.dt.float32
F32R = mybir.dt.float32r
F16 = mybir.dt.float16

B, T, D, H = 16, 2048, 1024, 64
N_CORES = 8
BPC = B // N_CORES          # batches per core
P_DROP = 0.2
NB = T // 128               # 16 key chunks per batch
NG = T // 512               # 4 query groups per batch
GROUP = 4                   # key chunks per query group
MASK_NEG = -30000.0         # causal mask addend (fp16-representable)


# walrus here allows only ONE sync-wait command per instruction; Tile can
# attach several (e.g. its exit drain). Move extras onto same-engine NOPs.
def _split_excess_waits(nc):
    n = 0
    for f in nc.m.functions:
        for bb in f.blocks:
            new_insts = []
            changed = False
            for inst in bb.instructions:
                si = inst.sync_info
                if si is not None and si.on_wait and len(si.on_wait) > 1:
                    waits = list(si.on_wait)
                    extra, keep = waits[:-1], waits[-1:]
                    for i, w in enumerate(extra):
                        new_insts.append(mybir.InstNoOp(
                            name=f"{inst.name}-ws-{i}",
                            engine=inst.engine, ins=[], outs=[],
                            sync_info=mybir.SyncInfo(on_wait=[w], on_update=[]),
                            text_hint="waitsplit", bass_nofuse=True))
                        n += 1
                    si.on_wait = keep
                    changed = True
                new_insts.append(inst)
            if changed:
                bb.instructions[:] = new_insts
    return n


def _build(ctx: ExitStack, tc: "tile.TileContext", xt, wqk, wv, kfp, kdp, out):
    nc = tc.nc
    AF = mybir.ActivationFunctionType
    OP = mybir.AluOpType

    cpool = ctx.enter_context(tc.tile_pool(name="const", bufs=1))
    xpool = ctx.enter_context(tc.tile_pool(name="xt", bufs=2))
    qkvpool = ctx.enter_context(tc.tile_pool(name="qkv", bufs=2))
    vtpool = ctx.enter_context(tc.tile_pool(name="vt", bufs=2))
    kfpool = ctx.enter_context(tc.tile_pool(name="kf", bufs=2))
    kdpool = ctx.enter_context(tc.tile_pool(name="kd", bufs=4))
    eppool = ctx.enter_context(tc.tile_pool(name="ep", bufs=3))
    edpool = ctx.enter_context(tc.tile_pool(name="ed", bufs=3))
    pppool = ctx.enter_context(tc.tile_pool(name="pp", bufs=3))
    pdpool = ctx.enter_context(tc.tile_pool(name="pd", bufs=3))
    otsbpool = ctx.enter_context(tc.tile_pool(name="otsb", bufs=2))
    onsbpool = ctx.enter_context(tc.tile_pool(name="onsb", bufs=2))
    outpool = ctx.enter_context(tc.tile_pool(name="outsb", bufs=4))
    rdpool = ctx.enter_context(tc.tile_pool(name="rd", bufs=2))

    # PSUM: Sp 2x[128,1024] = 4 banks (pairs, v-proj, v-stage),
    # Sd 3x[128,512] = 3 banks (qk-proj, diag scores, rdbc),
    # combo (ot rows 0:64 + denom row 64) 1x[128,512] = 1 bank -> 8 total.
    pspool = ctx.enter_context(tc.tile_pool(name="psp", bufs=2, space="PSUM"))
    psdool = ctx.enter_context(tc.tile_pool(name="psd", bufs=3, space="PSUM"))
    combops = ctx.enter_context(tc.tile_pool(name="combo", bufs=1, space="PSUM"))

    # deferred group epilogues: each entry is a closure that emits the
    # PE/DVE tail of a finished group; flushed between the next group's
    # producers so the tensor engine never idles in the 1/d dependency
    # chain.
    eptail = []

    def flush_eptail():
        while eptail:
            eptail.pop(0)()

    # ---- constants -------------------------------------------------------
    ident_h = cpool.tile([128, 128], F16)
    make_identity(nc, ident_h[:])

    ones_h = cpool.tile([128, 1], F16)
    nc.gpsimd.memset(ones_h[:], 1.0)
    ones64_h = cpool.tile([1, 64], F16)
    nc.gpsimd.memset(ones64_h[:], 1.0)

    wqk_sb = cpool.tile([128, 8 * 128], F16)
    nc.sync.dma_start(
        wqk_sb[:].rearrange("p (c h) -> p c h", c=8),
        wqk.rearrange("(c p) h -> p c h", p=128))
    wv_sb = cpool.tile([128, 8 * H], F16)
    nc.sync.dma_start(
        wv_sb[:].rearrange("p (c h) -> p c h", c=8),
        wv.rearrange("(c p) h -> p c h", p=128))

    # staged x prefetch from the host-pre-tiled layout: each load is 128
    # contiguous 8KB descriptors (one per partition)
    xbigs = {}

    def load_x(b, quarter):
        if b >= BPC or quarter >= 4 or (b, quarter) in xbigs:
            return
        xb = xpool.tile([128, 8 * 512], F16, tag=f"x{b}{quarter}", bufs=1)
        nc.gpsimd.dma_start(xb[:], xt[b, quarter])
        xbigs[(b, quarter)] = xb

    # diag keep-mask prefetch: ONE packed DMA per group (cols:
    # 512 | 384 | 256 | 128 for the four diagonal blocks)
    KD_OFF = [0, 512, 896, 1152]
    kds = {}

    def load_kd(b, g):
        if b >= BPC or g >= NG or (b, g) in kds:
            return
        kd = kdpool.tile([128, 1280], F16, tag="kd", bufs=3)
        nc.sync.dma_start(kd[:], kdp[b, g])
        kds[(b, g)] = kd

    load_x(0, 0)
    load_x(0, 1)
    load_kd(0, 0)

    for b in range(BPC):
        # ---- phase A: projections ---------------------------------------
        # qkT[0:64,:] = q^T, qkT[64:128,:] = k^T ; v natural [s, H] tiles
        qkT = qkvpool.tile([128, T], F16, tag="qkT")
        kT0 = qkvpool.tile([64, T], F16, tag="kT0")
        vT = vtpool.tile([64, T], F16, tag="vT")
        v_sb = qkvpool.tile([128, NB * H], F16, tag="v")

        for quarter in range(4):
            if quarter + 2 < 4:
                load_x(b, quarter + 2)
            else:
                load_x(b + 1, quarter + 2 - 4)
            col = 512 * quarter
            xb = xbigs.pop((b, quarter))
            ps = psdool.tile([128, 512], F32, tag="Sd")
            for c in range(8):
                nc.tensor.matmul(
                    ps[:], wqk_sb[:, 128 * c:128 * (c + 1)],
                    xb[:, 512 * c:512 * (c + 1)],
                    start=(c == 0), stop=(c == 7))
            nc.vector.tensor_copy(qkT[:, col:col + 512], ps[:])
            # matmul needs lhsT/rhs at the same base partition: move k^T
            # (psum rows 64..127) down to partitions 0..63 via DMA. On the
            # Act HWDGE ring so it never queues behind bulk keep loads.
            nc.scalar.dma_start(kT0[:, col:col + 512], qkT[64:128, col:col + 512])
            ps2 = pspool.tile([64, 512], F32, tag="Sp",
                              padded_shape=[128, 1024])
            for c in range(8):
                nc.tensor.matmul(
                    ps2[:], wv_sb[:, H * c:H * (c + 1)],
                    xb[:, 512 * c:512 * (c + 1)],
                    start=(c == 0), stop=(c == 7))
            nc.scalar.copy(vT[:, col:col + 512], ps2[:])
            if quarter == 0:
                flush_eptail()  # prev batch's last-group tail
        qT = qkT
        kT = kT0

        # v: [64,T] -> natural [s, H] tiles, 8 PE transposes per PSUM bank
        for m in range(2):
            stage = pspool.tile([128, 512], F16, tag="Sp",
                                padded_shape=[128, 2048])
            for tloc in range(8):
                t = 8 * m + tloc
                nc.tensor.transpose(
                    stage[:, H * tloc:H * (tloc + 1)],
                    vT[:, 128 * t:128 * (t + 1)], ident_h[:64, :64])
            nc.vector.tensor_copy(
                v_sb[:, H * 8 * m:H * 8 * (m + 1)], stage[:])

        # ---- phase B: attention, per query group of 512 ------------------
        kfs = {}
        KF_OFF = {1: 0, 2: 2048, 3: 6144}

        def load_kf(g):
            if g > 3 or g == 0:
                return
            nf = 4 * g
            kf = kfpool.tile([128, nf * 512], F16, tag="kf",
                             padded_shape=[128, 12 * 512])
            nc.sync.dma_start(
                kf[:, :nf * 512],
                kfp[b, :, KF_OFF[g]:KF_OFF[g] + nf * 512])
            kfs[g] = kf

        load_kf(1)
        for g in range(NG):
            if g >= 1:
                load_kf(g + 1)
            if g + 1 < NG:
                load_kd(b, g + 1)
            else:
                load_kd(b + 1, 0)
            qcol = 512 * g
            kd_pack = kds.pop((b, g))
            kf = kfs.pop(g, None)
            combo = combops.tile([128, 512], F32, tag="combo")
            ot = combo[0:64, :]
            dps = combo[64:65, :]

            # work items: pairs of full chunks, then the 4 diagonal chunks
            items = [("pair", 2 * i) for i in range(2 * g)]
            items += [("diag", t) for t in range(4 * g, 4 * g + 4)]
            n_items = len(items)
            prod = {}

            def produce(i):
                kind, t = items[i]
                if kind == "pair":
                    sp = pspool.tile([128, 1024], F32, tag="Sp")
                    nc.tensor.matmul(
                        sp[:, 0:512], kT[:, 128 * t:128 * (t + 1)],
                        qT[0:64, qcol:qcol + 512], start=True, stop=True)
                    nc.tensor.matmul(
                        sp[:, 512:1024], kT[:, 128 * (t + 1):128 * (t + 2)],
                        qT[0:64, qcol:qcol + 512], start=True, stop=True)
                    E = eppool.tile([128, 1024], F16, tag="Ep")
                    nc.scalar.activation(
                        E[:], sp[:], AF.Exp, scale=float(H) ** -0.5)
                    prod[i] = E
                else:
                    qo = 128 * (t - 4 * g)
                    sd = psdool.tile([128, 512], F32, tag="Sd")
                    nc.tensor.matmul(
                        sd[:, qo:512], kT[:, 128 * t:128 * (t + 1)],
                        qT[0:64, qcol + qo:qcol + 512],
                        start=True, stop=True)
                    E = edpool.tile([128, 512], F16, tag="Ed")
                    nc.scalar.activation(
                        E[:, qo:512], sd[:, qo:512], AF.Exp,
                        scale=float(H) ** -0.5)
                    # causal mask: zero E above the diagonal of the
                    # 128x128 diagonal block (Pool; keeps PE out of it)
                    nc.gpsimd.affine_select(
                        out=E[:, qo:qo + 128], in_=E[:, qo:qo + 128],
                        compare_op=OP.is_ge, fill=0.0,
                        base=0, pattern=[[1, 128]], channel_multiplier=-1)
                    prod[i] = E

            def consume(i):
                kind, t = items[i]
                if kind == "pair":
                    E = prod.pop(i)
                    # denominator contributions (pre-dropout), written to
                    # partition row 64 of the shared ot/dps bank
                    nc.tensor.matmul(
                        combo[64:65, :], ones_h[:], E[:, 0:512],
                        start=(i == 0), stop=False, skip_group_check=True)
                    nc.tensor.matmul(
                        combo[64:65, :], ones_h[:], E[:, 512:1024],
                        start=False, stop=False, skip_group_check=True)
                    Pp = pppool.tile([128, 1024], F16, tag="Pp")
                    nc.vector.tensor_mul(
                        Pp[:], kf[:, 512 * t:512 * (t + 2)], E[:])
                    nc.tensor.matmul(
                        ot[:], v_sb[:, H * t:H * (t + 1)], Pp[:, 0:512],
                        start=(i == 0), stop=False)
                    nc.tensor.matmul(
                        ot[:], v_sb[:, H * (t + 1):H * (t + 2)],
                        Pp[:, 512:1024],
                        start=False, stop=False)
                else:
                    qo = 128 * (t - 4 * g)
                    E = prod.pop(i)
                    nc.tensor.matmul(
                        combo[64:65, qo:512], ones_h[:], E[:, qo:512],
                        start=(i == 0), stop=(i == n_items - 1),
                        skip_group_check=True)
                    j = t - 4 * g
                    Pd = pdpool.tile([128, 512], F16, tag="Pd")
                    nc.vector.tensor_mul(
                        Pd[:, qo:512],
                        kd_pack[:, KD_OFF[j]:KD_OFF[j] + 512 - qo],
                        E[:, qo:512])
                    nc.tensor.matmul(
                        ot[:, qo:512], v_sb[:, H * t:H * (t + 1)],
                        Pd[:, qo:512],
                        start=(i == 0), stop=(i == n_items - 1))

            # software-pipelined: consumers trail producers by PD items and
            # are emitted in pairs, giving the tensor engine longer
            # back-to-back matmul runs. The previous group's epilogue tail
            # is flushed between the first producers so its PE ops land
            # when their inputs are long since ready.
            PD = 2
            pend = []
            for i in range(n_items):
                produce(i)
                if i in (1, 2) and eptail:
                    eptail.pop(0)()
                if i >= PD:
                    pend.append(i - PD)
                    if len(pend) == 2:
                        consume(pend[0])
                        consume(pend[1])
                        pend = []
            for i in pend:
                consume(i)
            for i in range(max(0, n_items - PD), n_items):
                consume(i)

            # ---- group epilogue -----------------------------------------
            # immediate part: free dps/ot quickly. 1/d as exp(-ln d) on
            # ScalarE (a [1,512] DVE reciprocal measured ~4us on HW).
            ln_d = rdpool.tile([1, 512], F32, tag="rdf")
            nc.scalar.activation(ln_d[:], combo[64:65, :], AF.Ln)
            rd = rdpool.tile([1, 512], F16, tag="rd")
            nc.scalar.activation(rd[:], ln_d[:], AF.Exp, scale=-1.0)
            ot_sb = otsbpool.tile([64, 512], F32, tag="otsb")
            nc.vector.tensor_copy(ot_sb[:], combo[0:64, :])

            def make_tail(b, g, rd, ot_sb):
                def tail_pe():
                    rdbc = psdool.tile([64, 512], F32, tag="Sd")
                    nc.tensor.matmul(rdbc[:], ones64_h[:], rd[:],
                                     start=True, stop=True)
                    on_sb = onsbpool.tile([64, 512], F16, tag="onsb")
                    nc.vector.tensor_mul(on_sb[:], ot_sb[:], rdbc[:])
                    tail_pe.on_sb = on_sb

                def tail_out():
                    on_sb = tail_pe.on_sb
                    stage = psdool.tile([128, 256], F16, tag="Sd",
                                        padded_shape=[128, 1024])
                    for cc in range(GROUP):
                        nc.tensor.transpose(
                            stage[:, 64 * cc:64 * (cc + 1)],
                            on_sb[:, 128 * cc:128 * (cc + 1)],
                            ident_h[:64, :64])
                    osb = outpool.tile([128, 256], F16, tag="osb")
                    nc.vector.tensor_copy(osb[:], stage[:])
                    # store with fp16->f32 cast via SWDGE
                    nc.gpsimd.dma_start(
                        out[b].rearrange("(c p) h -> p c h", p=128)
                           [:, GROUP * g:GROUP * (g + 1), :],
                        osb[:].rearrange("p (c h) -> p c h", c=GROUP))
                return [tail_pe, tail_out]

            eptail.extend(make_tail(b, g, rd, ot_sb))

    flush_eptail()


_CACHE = {}


def _get_nc():
    if "nc" not in _CACHE:
        nc = bass.Bass("TRN2", target_bir_lowering=False)
        # all inputs host-pre-tiled so every DMA is 128 contiguous
        # per-partition runs
        xt = nc.dram_tensor("xt", [BPC, 4, 128, 4096], F16,
                            kind="ExternalInput")
        wqk = nc.dram_tensor("wqk", [D, 128], F16, kind="ExternalInput")
        wv = nc.dram_tensor("wv", [D, H], F16, kind="ExternalInput")
        kfp = nc.dram_tensor("kfp", [BPC, 128, 12288], F16,
                             kind="ExternalInput")
        kdp = nc.dram_tensor("kdp", [BPC, 4, 128, 1280], F16,
                             kind="ExternalInput")
        out = nc.dram_tensor("out", [BPC, T, H], F32, kind="ExternalOutput")
        with tile.TileContext(nc) as tc:
            with ExitStack() as ctx:
                _build(ctx, tc, xt.ap(), wqk.ap(), wv.ap(), kfp.ap(),
                       kdp.ap(), out.ap())
        _split_excess_waits(nc)
        _CACHE["nc"] = nc
    return _CACHE["nc"]


def kernel(x, Wq, Wk, Wv, drop_u, _trace=False):
    x = np.asarray(x)
    drop_u = np.asarray(drop_u)

    nc = _get_nc()
    # x^T pre-tiled: xtp[b, q, p, 512c+tt] = x[b, 512q+tt, 128c+p]
    xt_full = x.astype(np.float16).transpose(0, 2, 1)      # [B, D, T]
    xtp = np.ascontiguousarray(
        xt_full.reshape(B, 8, 128, 4, 512).transpose(0, 3, 2, 1, 4)
        .reshape(B, 4, 128, 4096))
    keep = (drop_u >= np.float32(P_DROP))
    keep = (keep.astype(np.float16) * np.float16(1.0 / (1.0 - P_DROP)))
    keepT = keep.transpose(0, 2, 1)                        # [B, s, q]
    # full-chunk keep blocks, packed per group then per partition:
    # kfp[b, p, off_g + 512c + q] = keepT[b, 128c+p, 512g+q]
    kfp = np.empty((B, 128, 12288), np.float16)
    off = 0
    for g in (1, 2, 3):
        nf = 4 * g
        blk = keepT[:, 0:128 * nf, 512 * g:512 * (g + 1)]
        blk = (blk.reshape(B, nf, 128, 512).transpose(0, 2, 1, 3)
               .reshape(B, 128, nf * 512))
        kfp[:, :, off:off + nf * 512] = blk
        off += nf * 512
    # diagonal keep blocks packed per group: widths 512|384|256|128
    kdp = np.empty((B, 4, 128, 1280), np.float16)
    for g in range(4):
        off2 = 0
        for j in range(4):
            t = 4 * g + j
            qo = 128 * j
            w = 512 - qo
            kdp[:, g, :, off2:off2 + w] = \
                keepT[:, 128 * t:128 * (t + 1), 512 * g + qo:512 * (g + 1)]
            off2 += w
    wqk = np.ascontiguousarray(
        np.concatenate([np.asarray(Wq), np.asarray(Wk)], axis=1)
        .astype(np.float16))                               # [D, 128]
    wv16 = np.asarray(Wv).astype(np.float16)
    in_maps = []
    for c in range(N_CORES):
        lo = BPC * c
        in_maps.append({
            "xt": xtp[lo:lo + BPC],
            "wqk": wqk, "wv": wv16,
            "kfp": kfp[lo:lo + BPC],
            "kdp": kdp[lo:lo + BPC],
        })
    res = run_bass_kernel_spmd(
        nc, in_maps, core_ids=list(range(N_CORES)), trace=_trace)
    out = np.concatenate([res.results[c]["out"] for c in range(N_CORES)], axis=0)
    if _trace:
        kernel.last_exec_time_ns = res.exec_time_ns
        kernel.last_results = res
    return out


# revision 56
# speedup vs baseline: 1.1020x; 1.0958x over previous
"""Trainium2 Bass kernel for single-head causal attention with dropout.

reference:
    q,k,v = x@Wq, x@Wk, x@Wv          [B,T,H]
    wei = softmax(mask(q@k^T * H**-0.5))   (causal)
    wei = wei * (drop_u >= 0.2)/0.8
    out = wei @ v                      [B,T,H]

B=16, T=2048, D=1024, H=64. 8 NeuronCores, data-parallel over batch
(2 batches/core).

Design notes (v3):
- Everything on-chip is fp16 (matmuls 1 cyc/row at any size, half the
  HBM bytes for x). Accumulations in f32 PSUM.
- The dropout mask ships from host as an exact fp16 keep-mask
  {0, 1.25}; dropout is a plain DVE tensor_tensor multiply.
- Scores transposed S^T[s, q] in [128 x 512] chunks; full chunks
  processed in pairs sharing one 2-bank PSUM tile so a single exp
  covers 1024 columns. Causal mask via a bf-style -30000 add-matmul
  on the diagonal blocks (exp underflows to exact 0).
- Softmax denominator: per-chunk ones^T @ E matmuls accumulated in a
  [1,512] PSUM bank (PE cost is tiny vs DVE alternatives measured on
  HW); 1/d via ScalarE exp(-ln d) (DVE single-partition reciprocal
  measured 4us(!) per call).
- Output stores via gpsimd SWDGE with fp16->f32 cast on the fly.
- Group epilogues (1/d, normalize, transpose-out) are DEFERRED into
  the next group's pipeline so the tensor engine never sits in the
  ln->exp->rdbc dependency bubble; ot and dps share one 2-buffered
  PSUM bank to make that legal within 8 banks.
"""

import numpy as np
from contextlib import ExitStack


def _ensure_ntff_hook():
    """The agent image's `antenv` lacks `axon_hooks`, so trn_boot's NTFF
    profile hook registration degrades silently and trace=True dies on
    import. Provide the missing module + register the ctypes hook."""
    import sys, types
    try:
        from antenv.axon_hooks import get_axon_ntff_profile_hook  # noqa
        return  # real module present
    except ImportError:
        pass
    try:
        import antenv
        mod = types.ModuleType("antenv.axon_hooks")
        _holder = [None]
        mod.set_axon_ntff_profile_hook = lambda h: _holder.__setitem__(0, h)
        mod.get_axon_ntff_profile_hook = lambda: _holder[0]
        sys.modules["antenv.axon_hooks"] = mod
        antenv.axon_hooks = mod
        from trn_agent_boot.trn_boot import _ntff_profile_via_ctypes
        mod.set_axon_ntff_profile_hook(
            _ntff_profile_via_ctypes("/opt/axon/libaxon_pjrt.so"))
    except Exception:
        pass


_ensure_ntff_hook()

import concourse.bass as bass
import concourse.tile as tile
from concourse import mybir
from concourse.bass_utils import run_bass_kernel_spmd
from concourse.masks import make_identity

F32 = mybir.dt.float32
F32R = mybir.dt.float32r
F16 = mybir.dt.float16

B, T, D, H = 16, 2048, 1024, 64
N_CORES = 8
BPC = B // N_CORES          # batches per core
P_DROP = 0.2
NB = T // 128               # 16 key chunks per batch
NG = T // 512               # 4 query groups per batch
GROUP = 4                   # key chunks per query group
MASK_NEG = -30000.0         # causal mask addend (fp16-representable)


# walrus here allows only ONE sync-wait command per instruction; Tile can
# attach several (e.g. its exit drain). Move extras onto same-engine NOPs.
def _split_excess_waits(nc):
    n = 0
    for f in nc.m.functions:
        for bb in f.blocks:
            new_insts = []
            changed = False
            for inst in bb.instructions:
                si = inst.sync_info
                if si is not None and si.on_wait and len(si.on_wait) > 1:
                    waits = list(si.on_wait)
                    extra, keep = waits[:-1], waits[-1:]
                    for i, w in enumerate(extra):
                        new_insts.append(mybir.InstNoOp(
                            name=f"{inst.name}-ws-{i}",
                            engine=inst.engine, ins=[], outs=[],
                            sync_info=mybir.SyncInfo(on_wait=[w], on_update=[]),
                            text_hint="waitsplit", bass_nofuse=True))
                        n += 1
                    si.on_wait = keep
                    changed = True
                new_insts.append(inst)
            if changed:
                bb.instructions[:] = new_insts
    return n


def _build(ctx: ExitStack, tc: "tile.TileContext", xt, wqk, wv, kfp, kdp, out):
    nc = tc.nc
    AF = mybir.ActivationFunctionType
    OP = mybir.AluOpType

    cpool = ctx.enter_context(tc.tile_pool(name="const", bufs=1))
    xpool = ctx.enter_context(tc.tile_pool(name="xt", bufs=2))
    qkvpool = ctx.enter_context(tc.tile_pool(name="qkv", bufs=2))
    vtpool = ctx.enter_context(tc.tile_pool(name="vt", bufs=2))
    kfpool = ctx.enter_context(tc.tile_pool(name="kf", bufs=2))
    kdpool = ctx.enter_context(tc.tile_pool(name="kd", bufs=4))
    eppool = ctx.enter_context(tc.tile_pool(name="ep", bufs=3))
    edpool = ctx.enter_context(tc.tile_pool(name="ed", bufs=3))
    pppool = ctx.enter_context(tc.tile_pool(name="pp", bufs=3))
    pdpool = ctx.enter_context(tc.tile_pool(name="pd", bufs=3))
    otsbpool = ctx.enter_context(tc.tile_pool(name="otsb", bufs=2))
    onsbpool = ctx.enter_context(tc.tile_pool(name="onsb", bufs=2))
    outpool = ctx.enter_context(tc.tile_pool(name="outsb", bufs=4))
    rdpool = ctx.enter_context(tc.tile_pool(name="rd", bufs=2))

    # PSUM: Sp 2x[128,1024] = 4 banks (pairs, v-proj, v-stage),
    # Sd 3x[128,512] = 3 banks (qk-proj, diag scores, rdbc),
    # combo (ot rows 0:64 + denom row 64) 1x[128,512] = 1 bank -> 8 total.
    pspool = ctx.enter_context(tc.tile_pool(name="psp", bufs=2, space="PSUM"))
    psdool = ctx.enter_context(tc.tile_pool(name="psd", bufs=3, space="PSUM"))
    combops = ctx.enter_context(tc.tile_pool(name="combo", bufs=1, space="PSUM"))

    # deferred group epilogues: each entry is a closure that emits the
    # PE/DVE tail of a finished group; flushed between the next group's
    # producers so the tensor engine never idles in the 1/d dependency
    # chain.
    eptail = []

    def flush_eptail():
        while eptail:
            eptail.pop(0)()

    # ---- weights + first loads (emitted before const setup so the Pool
    # and SP rings start streaming immediately) --------------------------
    wqk_sb = cpool.tile([128, 8 * 128], F16)
    nc.sync.dma_start(wqk_sb[:], wqk)
    wv_sb = cpool.tile([128, 8 * H], F16)
    nc.sync.dma_start(wv_sb[:], wv)

    # staged x prefetch from the host-pre-tiled layout: each load is 128
    # contiguous 8KB descriptors (one per partition)
    xbigs = {}

    def load_x(b, quarter):
        if b >= BPC or quarter >= 4 or (b, quarter) in xbigs:
            return
        xb = xpool.tile([128, 8 * 512], F16, tag=f"x{b}{quarter}", bufs=1)
        nc.gpsimd.dma_start(xb[:], xt[b, quarter])
        xbigs[(b, quarter)] = xb

    # diag keep-mask prefetch: ONE packed DMA per group (cols:
    # 512 | 384 | 256 | 128 for the four diagonal blocks)
    KD_OFF = [0, 512, 896, 1152]
    kds = {}

    def load_kd(b, g):
        if b >= BPC or g >= NG or (b, g) in kds:
            return
        kd = kdpool.tile([128, 1280], F16, tag="kd", bufs=3)
        nc.sync.dma_start(kd[:], kdp[b, g])
        kds[(b, g)] = kd

    load_x(0, 0)
    load_x(0, 1)
    load_kd(0, 0)

    # ---- constants ------------------------------------------------------
    ident_h = cpool.tile([128, 128], F16)
    make_identity(nc, ident_h[:])
    ones_h = cpool.tile([128, 1], F16)
    nc.gpsimd.memset(ones_h[:], 1.0)
    ones64_h = cpool.tile([1, 64], F16)
    nc.gpsimd.memset(ones64_h[:], 1.0)

    for b in range(BPC):
        # ---- phase A: projections ---------------------------------------
        # qkT[0:64,:] = q^T, qkT[64:128,:] = k^T ; v natural [s, H] tiles
        qkT = qkvpool.tile([128, T], F16, tag="qkT")
        kT0 = qkvpool.tile([64, T], F16, tag="kT0")
        vT = vtpool.tile([64, T], F16, tag="vT")
        v_sb = qkvpool.tile([128, NB * H], F16, tag="v")

        for quarter in range(4):
            if quarter + 2 < 4:
                load_x(b, quarter + 2)
            else:
                load_x(b + 1, quarter + 2 - 4)
            col = 512 * quarter
            xb = xbigs.pop((b, quarter))
            ps = psdool.tile([128, 512], F32, tag="Sd")
            for c in range(8):
                nc.tensor.matmul(
                    ps[:], wqk_sb[:, 128 * c:128 * (c + 1)],
                    xb[:, 512 * c:512 * (c + 1)],
                    start=(c == 0), stop=(c == 7))
            nc.vector.tensor_copy(qkT[:, col:col + 512], ps[:])
            # matmul needs lhsT/rhs at the same base partition: move k^T
            # (psum rows 64..127) down to partitions 0..63 via DMA. On the
            # Act HWDGE ring so it never queues behind bulk keep loads.
            nc.scalar.dma_start(kT0[:, col:col + 512], qkT[64:128, col:col + 512])
            ps2 = pspool.tile([64, 512], F32, tag="Sp",
                              padded_shape=[128, 1024])
            for c in range(8):
                nc.tensor.matmul(
                    ps2[:], wv_sb[:, H * c:H * (c + 1)],
                    xb[:, 512 * c:512 * (c + 1)],
                    start=(c == 0), stop=(c == 7))
            nc.scalar.copy(vT[:, col:col + 512], ps2[:])
            if quarter == 0:
                flush_eptail()  # prev batch's last-group tail
        qT = qkT
        kT = kT0

        # v: [64,T] -> natural [s, H] tiles, 8 PE transposes per PSUM bank
        for m in range(2):
            stage = pspool.tile([128, 512], F16, tag="Sp",
                                padded_shape=[128, 2048])
            for tloc in range(8):
                t = 8 * m + tloc
                nc.tensor.transpose(
                    stage[:, H * tloc:H * (tloc + 1)],
                    vT[:, 128 * t:128 * (t + 1)], ident_h[:64, :64])
            nc.vector.tensor_copy(
                v_sb[:, H * 8 * m:H * 8 * (m + 1)], stage[:])

        # ---- phase B: attention, per query group of 512 ------------------
        kfs = {}
        KF_OFF = {1: 0, 2: 2048, 3: 6144}

        def load_kf(g):
            if g > 3 or g == 0:
                return
            nf = 4 * g
            kf = kfpool.tile([128, nf * 512], F16, tag="kf",
                             padded_shape=[128, 12 * 512])
            nc.sync.dma_start(
                kf[:, :nf * 512],
                kfp[b, :, KF_OFF[g]:KF_OFF[g] + nf * 512])
            kfs[g] = kf

        load_kf(1)
        for g in range(NG):
            if g >= 1:
                load_kf(g + 1)
            if g + 1 < NG:
                load_kd(b, g + 1)
            else:
                load_kd(b + 1, 0)
            qcol = 512 * g
            kd_pack = kds.pop((b, g))
            kf = kfs.pop(g, None)
            combo = combops.tile([128, 512], F32, tag="combo")
            ot = combo[0:64, :]
            dps = combo[64:65, :]

            # work items: pairs of full chunks, then the 4 diagonal chunks
            items = [("pair", 2 * i) for i in range(2 * g)]
            items += [("diag", t) for t in range(4 * g, 4 * g + 4)]
            n_items = len(items)
            prod = {}

            def produce(i):
                kind, t = items[i]
                if kind == "pair":
                    sp = pspool.tile([128, 1024], F32, tag="Sp")
                    nc.tensor.matmul(
                        sp[:, 0:512], kT[:, 128 * t:128 * (t + 1)],
                        qT[0:64, qcol:qcol + 512], start=True, stop=True)
                    nc.tensor.matmul(
                        sp[:, 512:1024], kT[:, 128 * (t + 1):128 * (t + 2)],
                        qT[0:64, qcol:qcol + 512], start=True, stop=True)
                    E = eppool.tile([128, 1024], F16, tag="Ep")
                    nc.scalar.activation(
                        E[:], sp[:], AF.Exp, scale=float(H) ** -0.5)
                    prod[i] = E
                else:
                    qo = 128 * (t - 4 * g)
                    sd = psdool.tile([128, 512], F32, tag="Sd")
                    nc.tensor.matmul(
                        sd[:, qo:512], kT[:, 128 * t:128 * (t + 1)],
                        qT[0:64, qcol + qo:qcol + 512],
                        start=True, stop=True)
                    E = edpool.tile([128, 512], F16, tag="Ed")
                    nc.scalar.activation(
                        E[:, qo:512], sd[:, qo:512], AF.Exp,
                        scale=float(H) ** -0.5)
                    # causal mask: zero E above the diagonal of the
                    # 128x128 diagonal block (Pool; keeps PE out of it)
                    nc.gpsimd.affine_select(
                        out=E[:, qo:qo + 128], in_=E[:, qo:qo + 128],
                        compare_op=OP.is_ge, fill=0.0,
                        base=0, pattern=[[1, 128]], channel_multiplier=-1)
                    prod[i] = E

            def consume(i):
                kind, t = items[i]
                if kind == "pair":
                    E = prod.pop(i)
                    # denominator contributions (pre-dropout), written to
                    # partition row 64 of the shared ot/dps bank
                    nc.tensor.matmul(
                        combo[64:65, :], ones_h[:], E[:, 0:512],
                        start=(i == 0), stop=False, skip_group_check=True)
                    nc.tensor.matmul(
                        combo[64:65, :], ones_h[:], E[:, 512:1024],
                        start=False, stop=False, skip_group_check=True)
                    Pp = pppool.tile([128, 1024], F16, tag="Pp")
                    nc.vector.tensor_mul(
                        Pp[:], kf[:, 512 * t:512 * (t + 2)], E[:])
                    nc.tensor.matmul(
                        ot[:], v_sb[:, H * t:H * (t + 1)], Pp[:, 0:512],
                        start=(i == 0), stop=False)
                    nc.tensor.matmul(
                        ot[:], v_sb[:, H * (t + 1):H * (t + 2)],
                        Pp[:, 512:1024],
                        start=False, stop=False)
                else:
                    qo = 128 * (t - 4 * g)
                    E = prod.pop(i)
                    nc.tensor.matmul(
                        combo[64:65, qo:512], ones_h[:], E[:, qo:512],
                        start=(i == 0), stop=(i == n_items - 1),
                        skip_group_check=True)
                    j = t - 4 * g
                    Pd = pdpool.tile([128, 512], F16, tag="Pd")
                    nc.vector.tensor_mul(
                        Pd[:, qo:512],
                        kd_pack[:, KD_OFF[j]:KD_OFF[j] + 512 - qo],
                        E[:, qo:512])
                    nc.tensor.matmul(
                        ot[:, qo:512], v_sb[:, H * t:H * (t + 1)],
                        Pd[:, qo:512],
                        start=(i == 0), stop=(i == n_items - 1))

            # software-pipelined: consumers trail producers by PD items and
            # are emitted in pairs, giving the tensor engine longer
            # back-to-back matmul runs. The previous group's epilogue tail
            # is flushed between the first producers so its PE ops land
            # when their inputs are long since ready.
            PD = 2
            pend = []
            for i in range(n_items):
                produce(i)
                if i in (1, 2) and eptail:
                    eptail.pop(0)()
                if i >= PD:
                    pend.append(i - PD)
                    if len(pend) == 2:
                        consume(pend[0])
                        consume(pend[1])
                        pend = []
            for i in pend:
                consume(i)
            for i in range(max(0, n_items - PD), n_items):
                consume(i)

            # ---- group epilogue -----------------------------------------
            # immediate part: free dps/ot quickly. 1/d as exp(-ln d) on
            # ScalarE (a [1,512] DVE reciprocal measured ~4us on HW).
            ln_d = rdpool.tile([1, 512], F32, tag="rdf")
            nc.scalar.activation(ln_d[:], combo[64:65, :], AF.Ln)
            ot_sb = otsbpool.tile([64, 512], F32, tag="otsb")
            nc.vector.tensor_copy(ot_sb[:], combo[0:64, :])

            def make_tail(b, g, ln_d, ot_sb):
                def tail_pe():
                    rd = rdpool.tile([1, 512], F16, tag="rd")
                    nc.scalar.activation(rd[:], ln_d[:], AF.Exp, scale=-1.0)
                    rdbc = psdool.tile([64, 512], F32, tag="Sd")
                    nc.tensor.matmul(rdbc[:], ones64_h[:], rd[:],
                                     start=True, stop=True)
                    on_sb = onsbpool.tile([64, 512], F16, tag="onsb")
                    nc.vector.tensor_mul(on_sb[:], ot_sb[:], rdbc[:])
                    tail_pe.on_sb = on_sb

                def tail_out():
                    on_sb = tail_pe.on_sb
                    stage = psdool.tile([128, 256], F16, tag="Sd",
                                        padded_shape=[128, 1024])
                    for cc in range(GROUP):
                        nc.tensor.transpose(
                            stage[:, 64 * cc:64 * (cc + 1)],
                            on_sb[:, 128 * cc:128 * (cc + 1)],
                            ident_h[:64, :64])
                    osb = outpool.tile([128, 256], F16, tag="osb")
                    nc.vector.tensor_copy(osb[:], stage[:])
                    # store with fp16->f32 cast via SWDGE
                    nc.gpsimd.dma_start(
                        out[b].rearrange("(c p) h -> p c h", p=128)
                           [:, GROUP * g:GROUP * (g + 1), :],
                        osb[:].rearrange("p (c h) -> p c h", c=GROUP))
                return [tail_pe, tail_out]

            eptail.extend(make_tail(b, g, ln_d, ot_sb))

    flush_eptail()


_CACHE = {}


def _get_nc():
    if "nc" not in _CACHE:
        nc = bass.Bass("TRN2", target_bir_lowering=False)
        # all inputs host-pre-tiled so every DMA is 128 contiguous
        # per-partition runs
        xt = nc.dram_tensor("xt", [BPC, 4, 128, 4096], F16,
                            kind="ExternalInput")
        wqk = nc.dram_tensor("wqk", [128, 8 * 128], F16, kind="ExternalInput")
        wv = nc.dram_tensor("wv", [128, 8 * H], F16, kind="ExternalInput")
        kfp = nc.dram_tensor("kfp", [BPC, 128, 12288], F16,
                             kind="ExternalInput")
        kdp = nc.dram_tensor("kdp", [BPC, 4, 128, 1280], F16,
                             kind="ExternalInput")
        out = nc.dram_tensor("out", [BPC, T, H], F32, kind="ExternalOutput")
        with tile.TileContext(nc) as tc:
            with ExitStack() as ctx:
                _build(ctx, tc, xt.ap(), wqk.ap(), wv.ap(), kfp.ap(),
                       kdp.ap(), out.ap())
        _split_excess_waits(nc)
        _CACHE["nc"] = nc
    return _CACHE["nc"]


def kernel(x, Wq, Wk, Wv, drop_u, _trace=False):
    x = np.asarray(x)
    drop_u = np.asarray(drop_u)

    nc = _get_nc()
    # x^T pre-tiled: xtp[b, q, p, 512c+tt] = x[b, 512q+tt, 128c+p]
    xt_full = x.astype(np.float16).transpose(0, 2, 1)      # [B, D, T]
    xtp = np.ascontiguousarray(
        xt_full.reshape(B, 8, 128, 4, 512).transpose(0, 3, 2, 1, 4)
        .reshape(B, 4, 128, 4096))
    keep = (drop_u >= np.float32(P_DROP))
    keep = (keep.astype(np.float16) * np.float16(1.0 / (1.0 - P_DROP)))
    keepT = keep.transpose(0, 2, 1)                        # [B, s, q]
    # full-chunk keep blocks, packed per group then per partition:
    # kfp[b, p, off_g + 512c + q] = keepT[b, 128c+p, 512g+q]
    kfp = np.empty((B, 128, 12288), np.float16)
    off = 0
    for g in (1, 2, 3):
        nf = 4 * g
        blk = keepT[:, 0:128 * nf, 512 * g:512 * (g + 1)]
        blk = (blk.reshape(B, nf, 128, 512).transpose(0, 2, 1, 3)
               .reshape(B, 128, nf * 512))
        kfp[:, :, off:off + nf * 512] = blk
        off += nf * 512
    # diagonal keep blocks packed per group: widths 512|384|256|128
    kdp = np.empty((B, 4, 128, 1280), np.float16)
    for g in range(4):
        off2 = 0
        for j in range(4):
            t = 4 * g + j
            qo = 128 * j
            w = 512 - qo
            kdp[:, g, :, off2:off2 + w] = \
                keepT[:, 128 * t:128 * (t + 1), 512 * g + qo:512 * (g + 1)]
            off2 += w
    # weights pre-tiled: w_sb[p, 128c+h] = W[128c+p, h]
    wqk0 = np.concatenate([np.asarray(Wq), np.asarray(Wk)],
                          axis=1).astype(np.float16)       # [D, 128]
    wqk = np.ascontiguousarray(
        wqk0.reshape(8, 128, 128).transpose(1, 0, 2).reshape(128, 1024))
    wv16 = np.ascontiguousarray(
        np.asarray(Wv).astype(np.float16)
        .reshape(8, 128, H).transpose(1, 0, 2).reshape(128, 8 * H))
    in_maps = []
    for c in range(N_CORES):
        lo = BPC * c
        in_maps.append({
            "xt": xtp[lo:lo + BPC],
            "wqk": wqk, "wv": wv16,
            "kfp": kfp[lo:lo + BPC],
            "kdp": kdp[lo:lo + BPC],
        })
    res = run_bass_kernel_spmd(
        nc, in_maps, core_ids=list(range(N_CORES)), trace=_trace)
    out = np.concatenate([res.results[c]["out"] for c in range(N_CORES)], axis=0)
    if _trace:
        kernel.last_exec_time_ns = res.exec_time_ns
        kernel.last_results = res
    return out


# revision 66
# speedup vs baseline: 1.2412x; 1.1263x over previous
"""Trainium2 Bass kernel for single-head causal attention with dropout.

reference:
    q,k,v = x@Wq, x@Wk, x@Wv          [B,T,H]
    wei = softmax(mask(q@k^T * H**-0.5))   (causal)
    wei = wei * (drop_u >= 0.2)/0.8
    out = wei @ v                      [B,T,H]

B=16, T=2048, D=1024, H=64. 8 NeuronCores, data-parallel over batch
(2 batches/core).

Design notes (v3):
- Everything on-chip is fp16 (matmuls 1 cyc/row at any size, half the
  HBM bytes for x). Accumulations in f32 PSUM.
- The dropout mask ships from host as an exact fp16 keep-mask
  {0, 1.25}; dropout is a plain DVE tensor_tensor multiply.
- Scores transposed S^T[s, q] in [128 x 512] chunks; full chunks
  processed in pairs sharing one 2-bank PSUM tile so a single exp
  covers 1024 columns. Causal mask via a bf-style -30000 add-matmul
  on the diagonal blocks (exp underflows to exact 0).
- Softmax denominator: per-chunk ones^T @ E matmuls accumulated in a
  [1,512] PSUM bank (PE cost is tiny vs DVE alternatives measured on
  HW); 1/d via ScalarE exp(-ln d) (DVE single-partition reciprocal
  measured 4us(!) per call).
- Output stores via gpsimd SWDGE with fp16->f32 cast on the fly.
- Group epilogues (1/d, normalize, transpose-out) are DEFERRED into
  the next group's pipeline so the tensor engine never sits in the
  ln->exp->rdbc dependency bubble; ot and dps share one 2-buffered
  PSUM bank to make that legal within 8 banks.
"""

import numpy as np
from contextlib import ExitStack


def _ensure_ntff_hook():
    """The agent image's `antenv` lacks `axon_hooks`, so trn_boot's NTFF
    profile hook registration degrades silently and trace=True dies on
    import. Provide the missing module + register the ctypes hook."""
    import sys, types
    try:
        from antenv.axon_hooks import get_axon_ntff_profile_hook  # noqa
        return  # real module present
    except ImportError:
        pass
    try:
        import antenv
        mod = types.ModuleType("antenv.axon_hooks")
        _holder = [None]
        mod.set_axon_ntff_profile_hook = lambda h: _holder.__setitem__(0, h)
        mod.get_axon_ntff_profile_hook = lambda: _holder[0]
        sys.modules["antenv.axon_hooks"] = mod
        antenv.axon_hooks = mod
        from trn_agent_boot.trn_boot import _ntff_profile_via_ctypes
        mod.set_axon_ntff_profile_hook(
            _ntff_profile_via_ctypes("/opt/axon/libaxon_pjrt.so"))
    except Exception:
        pass


_ensure_ntff_hook()

import concourse.bass as bass
import concourse.tile as tile
from concourse import mybir
from concourse.bass_utils import run_bass_kernel_spmd
from concourse.masks import make_identity

F32 = mybir.dt.float32
F32R = mybir.dt.float32r
F16 = mybir.dt.float16

B, T, D, H = 16, 2048, 1024, 64
N_CORES = 8
BPC = B // N_CORES          # batches per core
P_DROP = 0.2
NB = T // 128               # 16 key chunks per batch
NG = T // 512               # 4 query groups per batch
GROUP = 4                   # key chunks per query group
MASK_NEG = -30000.0         # causal mask addend (fp16-representable)


# walrus here allows only ONE sync-wait command per instruction; Tile can
# attach several (e.g. its exit drain). Move extras onto same-engine NOPs.
def _split_excess_waits(nc):
    n = 0
    for f in nc.m.functions:
        for bb in f.blocks:
            new_insts = []
            changed = False
            for inst in bb.instructions:
                si = inst.sync_info
                if si is not None and si.on_wait and len(si.on_wait) > 1:
                    waits = list(si.on_wait)
                    extra, keep = waits[:-1], waits[-1:]
                    for i, w in enumerate(extra):
                        new_insts.append(mybir.InstNoOp(
                            name=f"{inst.name}-ws-{i}",
                            engine=inst.engine, ins=[], outs=[],
                            sync_info=mybir.SyncInfo(on_wait=[w], on_update=[]),
                            text_hint="waitsplit", bass_nofuse=True))
                        n += 1
                    si.on_wait = keep
                    changed = True
                new_insts.append(inst)
            if changed:
                bb.instructions[:] = new_insts
    return n


def _build(ctx: ExitStack, tc: "tile.TileContext", xt, wqk, wv, kfp, kdp, out):
    nc = tc.nc
    AF = mybir.ActivationFunctionType
    OP = mybir.AluOpType

    cpool = ctx.enter_context(tc.tile_pool(name="const", bufs=1))
    xpool = ctx.enter_context(tc.tile_pool(name="xt", bufs=2))
    qkvpool = ctx.enter_context(tc.tile_pool(name="qkv", bufs=2))
    vtpool = ctx.enter_context(tc.tile_pool(name="vt", bufs=2))
    kfpool = ctx.enter_context(tc.tile_pool(name="kf", bufs=2))
    kdpool = ctx.enter_context(tc.tile_pool(name="kd", bufs=4))
    eppool = ctx.enter_context(tc.tile_pool(name="ep", bufs=3))
    edpool = ctx.enter_context(tc.tile_pool(name="ed", bufs=3))
    pppool = ctx.enter_context(tc.tile_pool(name="pp", bufs=3))
    pdpool = ctx.enter_context(tc.tile_pool(name="pd", bufs=3))
    otsbpool = ctx.enter_context(tc.tile_pool(name="otsb", bufs=2))
    onsbpool = ctx.enter_context(tc.tile_pool(name="onsb", bufs=2))
    outpool = ctx.enter_context(tc.tile_pool(name="outsb", bufs=4))
    rdpool = ctx.enter_context(tc.tile_pool(name="rd", bufs=2))

    # PSUM: Sp 2x[128,1024] = 4 banks (pairs, v-proj, v-stage),
    # Sd 2x[128,512] = 2 banks (qk-proj, diag scores, rdbc),
    # combo (ot rows 0:64 + denom rows 64:128) 2x[128,512] -> 8 total.
    pspool = ctx.enter_context(tc.tile_pool(name="psp", bufs=2, space="PSUM"))
    psdool = ctx.enter_context(tc.tile_pool(name="psd", bufs=2, space="PSUM"))
    combops = ctx.enter_context(tc.tile_pool(name="combo", bufs=2, space="PSUM"))

    # deferred group epilogues: each entry is a closure that emits the
    # PE/DVE tail of a finished group; flushed between the next group's
    # producers so the tensor engine never idles in the 1/d dependency
    # chain.
    eptail = []

    def flush_eptail():
        while eptail:
            eptail.pop(0)()

    # ---- weights + first loads (emitted before const setup so the Pool
    # and SP rings start streaming immediately) --------------------------
    wqk_sb = cpool.tile([128, 8 * 128], F16)
    nc.sync.dma_start(wqk_sb[:], wqk)
    wv_sb = cpool.tile([128, 8 * H], F16)
    nc.sync.dma_start(wv_sb[:], wv)

    # staged x prefetch from the host-pre-tiled layout: each load is 128
    # contiguous 8KB descriptors (one per partition)
    xbigs = {}

    def load_x(b, quarter):
        if b >= BPC or quarter >= 4 or (b, quarter) in xbigs:
            return
        xb = xpool.tile([128, 8 * 512], F16, tag=f"x{b}{quarter}", bufs=1)
        nc.gpsimd.dma_start(xb[:], xt[b, quarter])
        xbigs[(b, quarter)] = xb

    # diag keep-mask prefetch: ONE packed DMA per group (cols:
    # 512 | 384 | 256 | 128 for the four diagonal blocks)
    KD_OFF = [0, 512, 896, 1152]
    kds = {}

    def load_kd(b, g):
        if b >= BPC or g >= NG or (b, g) in kds:
            return
        kd = kdpool.tile([128, 1280], F16, tag="kd", bufs=3)
        nc.sync.dma_start(kd[:], kdp[b, g])
        kds[(b, g)] = kd

    load_x(0, 0)
    load_x(0, 1)
    load_kd(0, 0)

    # ---- constants ------------------------------------------------------
    ident_h = cpool.tile([128, 128], F16)
    make_identity(nc, ident_h[:])
    ones_w = cpool.tile([128, 64], F16)
    nc.gpsimd.memset(ones_w[:], 1.0)
    ones64_h = cpool.tile([1, 64], F16)
    nc.gpsimd.memset(ones64_h[:], 1.0)

    for b in range(BPC):
        # ---- phase A: projections ---------------------------------------
        # qkT[0:64,:] = q^T, qkT[64:128,:] = k^T ; v natural [s, H] tiles
        qkT = qkvpool.tile([128, T], F16, tag="qkT")
        # kT0 rows 0:63 = k^T; rows 64:127 stay ZERO so the scores
        # matmul can run with K=128 (measured faster than K=64) against
        # the full qkT as rhs -- the zero weight rows null out the k^T
        # half of qkT.
        kT0 = qkvpool.tile([128, T], F16, tag="kT0")
        nc.gpsimd.memset(kT0[64:128, :], 0.0)
        vT = vtpool.tile([64, T], F16, tag="vT")
        v_sb = qkvpool.tile([128, NB * H], F16, tag="v")

        for quarter in range(4):
            if quarter + 2 < 4:
                load_x(b, quarter + 2)
            else:
                load_x(b + 1, quarter + 2 - 4)
            col = 512 * quarter
            xb = xbigs.pop((b, quarter))
            ps = psdool.tile([128, 512], F32, tag="Sd")
            for c in range(8):
                nc.tensor.matmul(
                    ps[:], wqk_sb[:, 128 * c:128 * (c + 1)],
                    xb[:, 512 * c:512 * (c + 1)],
                    start=(c == 0), stop=(c == 7))
            nc.vector.tensor_copy(qkT[:, col:col + 512], ps[:])
            # matmul needs lhsT/rhs at the same base partition: move k^T
            # (psum rows 64..127) down to partitions 0..63 via DMA. On the
            # Act HWDGE ring so it never queues behind bulk keep loads.
            nc.scalar.dma_start(kT0[0:64, col:col + 512],
                                qkT[64:128, col:col + 512])
            ps2 = pspool.tile([64, 512], F32, tag="Sp",
                              padded_shape=[128, 1024])
            for c in range(8):
                nc.tensor.matmul(
                    ps2[:], wv_sb[:, H * c:H * (c + 1)],
                    xb[:, 512 * c:512 * (c + 1)],
                    start=(c == 0), stop=(c == 7))
            nc.scalar.copy(vT[:, col:col + 512], ps2[:])
            if quarter == 0:
                flush_eptail()  # prev batch's last-group tail
        qT = qkT
        kT = kT0

        # v: [64,T] -> natural [s, H] tiles, 8 PE transposes per PSUM bank
        for m in range(2):
            stage = pspool.tile([128, 512], F16, tag="Sp",
                                padded_shape=[128, 2048])
            for tloc in range(8):
                t = 8 * m + tloc
                nc.tensor.transpose(
                    stage[:, H * tloc:H * (tloc + 1)],
                    vT[:, 128 * t:128 * (t + 1)], ident_h[:64, :64])
            nc.vector.tensor_copy(
                v_sb[:, H * 8 * m:H * 8 * (m + 1)], stage[:])

        # ---- phase B: attention, per query group of 512 ------------------
        kfs = {}
        KF_OFF = {1: 0, 2: 2048, 3: 6144}

        def load_kf(g):
            if g > 3 or g == 0:
                return
            nf = 4 * g
            kf = kfpool.tile([128, nf * 512], F16, tag="kf",
                             padded_shape=[128, 12 * 512])
            nc.sync.dma_start(
                kf[:, :nf * 512],
                kfp[b, :, KF_OFF[g]:KF_OFF[g] + nf * 512])
            kfs[g] = kf

        load_kf(1)
        for g in range(NG):
            if g >= 1:
                load_kf(g + 1)
            if g + 1 < NG:
                load_kd(b, g + 1)
            else:
                load_kd(b + 1, 0)
            qcol = 512 * g
            kd_pack = kds.pop((b, g))
            kf = kfs.pop(g, None)
            combo = combops.tile([128, 512], F32, tag="combo")
            ot = combo[0:64, :]
            dps = combo[64:65, :]

            # work items: pairs of full chunks, then the 4 diagonal chunks
            items = [("pair", 2 * i) for i in range(2 * g)]
            items += [("diag", t) for t in range(4 * g, 4 * g + 4)]
            n_items = len(items)
            prod = {}

            def produce(i):
                kind, t = items[i]
                if kind == "pair":
                    sp = pspool.tile([128, 1024], F32, tag="Sp")
                    nc.tensor.matmul(
                        sp[:, 0:512], kT[:, 128 * t:128 * (t + 1)],
                        qT[:, qcol:qcol + 512], start=True, stop=True)
                    nc.tensor.matmul(
                        sp[:, 512:1024], kT[:, 128 * (t + 1):128 * (t + 2)],
                        qT[:, qcol:qcol + 512], start=True, stop=True)
                    E = eppool.tile([128, 1024], F16, tag="Ep")
                    nc.scalar.activation(
                        E[:], sp[:], AF.Exp, scale=float(H) ** -0.5)
                    prod[i] = E
                else:
                    qo = 128 * (t - 4 * g)
                    sd = psdool.tile([128, 512], F32, tag="Sd")
                    nc.tensor.matmul(
                        sd[:, qo:512], kT[:, 128 * t:128 * (t + 1)],
                        qT[:, qcol + qo:qcol + 512],
                        start=True, stop=True)
                    E = edpool.tile([128, 512], F16, tag="Ed")
                    nc.scalar.activation(
                        E[:, qo:512], sd[:, qo:512], AF.Exp,
                        scale=float(H) ** -0.5)
                    # causal mask: zero E above the diagonal of the
                    # 128x128 diagonal block (Pool; keeps PE out of it)
                    nc.gpsimd.affine_select(
                        out=E[:, qo:qo + 128], in_=E[:, qo:qo + 128],
                        compare_op=OP.is_ge, fill=0.0,
                        base=0, pattern=[[1, 128]], channel_multiplier=-1)
                    prod[i] = E

            def consume(i):
                kind, t = items[i]
                if kind == "pair":
                    E = prod.pop(i)
                    # denominator contributions (pre-dropout), written as
                    # 64 duplicate rows to partitions 64:128 of the shared
                    # ot/dps bank (M=1 matmuls measured ~110ns slower)
                    nc.tensor.matmul(
                        combo[64:128, :], ones_w[:], E[:, 0:512],
                        start=(i == 0), stop=False, skip_group_check=True)
                    nc.tensor.matmul(
                        combo[64:128, :], ones_w[:], E[:, 512:1024],
                        start=False, stop=False, skip_group_check=True)
                    Pp = pppool.tile([128, 1024], F16, tag="Pp")
                    nc.vector.tensor_mul(
                        Pp[:], kf[:, 512 * t:512 * (t + 2)], E[:])
                    nc.tensor.matmul(
                        ot[:], v_sb[:, H * t:H * (t + 1)], Pp[:, 0:512],
                        start=(i == 0), stop=False)
                    nc.tensor.matmul(
                        ot[:], v_sb[:, H * (t + 1):H * (t + 2)],
                        Pp[:, 512:1024],
                        start=False, stop=False)
                else:
                    qo = 128 * (t - 4 * g)
                    E = prod.pop(i)
                    nc.tensor.matmul(
                        combo[64:128, qo:512], ones_w[:], E[:, qo:512],
                        start=(i == 0), stop=(i == n_items - 1),
                        skip_group_check=True)
                    j = t - 4 * g
                    Pd = pdpool.tile([128, 512], F16, tag="Pd")
                    nc.vector.tensor_mul(
                        Pd[:, qo:512],
                        kd_pack[:, KD_OFF[j]:KD_OFF[j] + 512 - qo],
                        E[:, qo:512])
                    nc.tensor.matmul(
                        ot[:, qo:512], v_sb[:, H * t:H * (t + 1)],
                        Pd[:, qo:512],
                        start=(i == 0), stop=(i == n_items - 1))

            # software-pipelined: consumers trail producers by PD items and
            # are emitted in pairs, giving the tensor engine longer
            # back-to-back matmul runs. The previous group's epilogue tail
            # is flushed between the first producers so its PE ops land
            # when their inputs are long since ready.
            PD = 2
            pend = []
            for i in range(n_items):
                produce(i)
                if i in (1, 2) and eptail:
                    eptail.pop(0)()
                if i >= PD:
                    pend.append(i - PD)
                    if len(pend) == 2:
                        consume(pend[0])
                        consume(pend[1])
                        pend = []
            for i in pend:
                consume(i)
            for i in range(max(0, n_items - PD), n_items):
                consume(i)

            # ---- group epilogue -----------------------------------------
            # immediate part: free dps/ot quickly. 1/d as exp(-ln d) on
            # ScalarE (a [1,512] DVE reciprocal measured ~4us on HW).
            ot_sb = otsbpool.tile([64, 512], F32, tag="otsb")
            nc.vector.tensor_copy(ot_sb[:], combo[0:64, :])

            def make_tail(b, g, combo, ot_sb):
                def tail_pe():
                    ln_d = rdpool.tile([1, 512], F32, tag="rdf")
                    nc.scalar.activation(ln_d[:], combo[64:65, :], AF.Ln)
                    rd = rdpool.tile([1, 512], F16, tag="rd")
                    nc.scalar.activation(rd[:], ln_d[:], AF.Exp, scale=-1.0)
                    rdbc = psdool.tile([64, 512], F32, tag="Sd")
                    nc.tensor.matmul(rdbc[:], ones64_h[:], rd[:],
                                     start=True, stop=True)
                    on_sb = onsbpool.tile([64, 512], F16, tag="onsb")
                    nc.vector.tensor_mul(on_sb[:], ot_sb[:], rdbc[:])
                    tail_pe.on_sb = on_sb

                def tail_out():
                    on_sb = tail_pe.on_sb
                    stage = psdool.tile([128, 256], F16, tag="Sd",
                                        padded_shape=[128, 1024])
                    for cc in range(GROUP):
                        nc.tensor.transpose(
                            stage[:, 64 * cc:64 * (cc + 1)],
                            on_sb[:, 128 * cc:128 * (cc + 1)],
                            ident_h[:64, :64])
                    osb = outpool.tile([128, 256], F16, tag="osb")
                    nc.vector.tensor_copy(osb[:], stage[:])
                    # store with fp16->f32 cast via SWDGE
                    nc.gpsimd.dma_start(
                        out[b].rearrange("(c p) h -> p c h", p=128)
                           [:, GROUP * g:GROUP * (g + 1), :],
                        osb[:].rearrange("p (c h) -> p c h", c=GROUP))
                return [tail_pe, tail_out]

            eptail.extend(make_tail(b, g, combo, ot_sb))

    flush_eptail()


_CACHE = {}


def _get_nc():
    if "nc" not in _CACHE:
        nc = bass.Bass("TRN2", target_bir_lowering=False)
        # all inputs host-pre-tiled so every DMA is 128 contiguous
        # per-partition runs
        xt = nc.dram_tensor("xt", [BPC, 4, 128, 4096], F16,
                            kind="ExternalInput")
        wqk = nc.dram_tensor("wqk", [128, 8 * 128], F16, kind="ExternalInput")
        wv = nc.dram_tensor("wv", [128, 8 * H], F16, kind="ExternalInput")
        kfp = nc.dram_tensor("kfp", [BPC, 128, 12288], F16,
                             kind="ExternalInput")
        kdp = nc.dram_tensor("kdp", [BPC, 4, 128, 1280], F16,
                             kind="ExternalInput")
        out = nc.dram_tensor("out", [BPC, T, H], F32, kind="ExternalOutput")
        with tile.TileContext(nc) as tc:
            with ExitStack() as ctx:
                _build(ctx, tc, xt.ap(), wqk.ap(), wv.ap(), kfp.ap(),
                       kdp.ap(), out.ap())
        _split_excess_waits(nc)
        _CACHE["nc"] = nc
    return _CACHE["nc"]


def kernel(x, Wq, Wk, Wv, drop_u, _trace=False):
    x = np.asarray(x)
    drop_u = np.asarray(drop_u)

    nc = _get_nc()
    # x^T pre-tiled: xtp[b, q, p, 512c+tt] = x[b, 512q+tt, 128c+p]
    xt_full = x.astype(np.float16).transpose(0, 2, 1)      # [B, D, T]
    xtp = np.ascontiguousarray(
        xt_full.reshape(B, 8, 128, 4, 512).transpose(0, 3, 2, 1, 4)
        .reshape(B, 4, 128, 4096))
    keep = (drop_u >= np.float32(P_DROP))
    keep = (keep.astype(np.float16) * np.float16(1.0 / (1.0 - P_DROP)))
    keepT = keep.transpose(0, 2, 1)                        # [B, s, q]
    # full-chunk keep blocks, packed per group then per partition:
    # kfp[b, p, off_g + 512c + q] = keepT[b, 128c+p, 512g+q]
    kfp = np.empty((B, 128, 12288), np.float16)
    off = 0
    for g in (1, 2, 3):
        nf = 4 * g
        blk = keepT[:, 0:128 * nf, 512 * g:512 * (g + 1)]
        blk = (blk.reshape(B, nf, 128, 512).transpose(0, 2, 1, 3)
               .reshape(B, 128, nf * 512))
        kfp[:, :, off:off + nf * 512] = blk
        off += nf * 512
    # diagonal keep blocks packed per group: widths 512|384|256|128
    kdp = np.empty((B, 4, 128, 1280), np.float16)
    for g in range(4):
        off2 = 0
        for j in range(4):
            t = 4 * g + j
            qo = 128 * j
            w = 512 - qo
            kdp[:, g, :, off2:off2 + w] = \
                keepT[:, 128 * t:128 * (t + 1), 512 * g + qo:512 * (g + 1)]
            off2 += w
    # weights pre-tiled: w_sb[p, 128c+h] = W[128c+p, h]
    wqk0 = np.concatenate([np.asarray(Wq), np.asarray(Wk)],
                          axis=1).astype(np.float16)       # [D, 128]
    wqk = np.ascontiguousarray(
        wqk0.reshape(8, 128, 128).transpose(1, 0, 2).reshape(128, 1024))
    wv16 = np.ascontiguousarray(
        np.asarray(Wv).astype(np.float16)
        .reshape(8, 128, H).transpose(1, 0, 2).reshape(128, 8 * H))
    in_maps = []
    for c in range(N_CORES):
        lo = BPC * c
        in_maps.append({
            "xt": xtp[lo:lo + BPC],
            "wqk": wqk, "wv": wv16,
            "kfp": kfp[lo:lo + BPC],
            "kdp": kdp[lo:lo + BPC],
        })
    res = run_bass_kernel_spmd(
        nc, in_maps, core_ids=list(range(N_CORES)), trace=_trace)
    out = np.concatenate([res.results[c]["out"] for c in range(N_CORES)], axis=0)
    if _trace:
        kernel.last_exec_time_ns = res.exec_time_ns
        kernel.last_results = res
    return out


# revision 68
# speedup vs baseline: 1.2690x; 1.0224x over previous
"""Trainium2 Bass kernel for single-head causal attention with dropout.

reference:
    q,k,v = x@Wq, x@Wk, x@Wv          [B,T,H]
    wei = softmax(mask(q@k^T * H**-0.5))   (causal)
    wei = wei * (drop_u >= 0.2)/0.8
    out = wei @ v                      [B,T,H]

B=16, T=2048, D=1024, H=64. 8 NeuronCores, data-parallel over batch
(2 batches/core).

Design notes (v3):
- Everything on-chip is fp16 (matmuls 1 cyc/row at any size, half the
  HBM bytes for x). Accumulations in f32 PSUM.
- The dropout mask ships from host as an exact fp16 keep-mask
  {0, 1.25}; dropout is a plain DVE tensor_tensor multiply.
- Scores transposed S^T[s, q] in [128 x 512] chunks; full chunks
  processed in pairs sharing one 2-bank PSUM tile so a single exp
  covers 1024 columns. Causal mask via a bf-style -30000 add-matmul
  on the diagonal blocks (exp underflows to exact 0).
- Softmax denominator: per-chunk ones^T @ E matmuls accumulated in a
  [1,512] PSUM bank (PE cost is tiny vs DVE alternatives measured on
  HW); 1/d via ScalarE exp(-ln d) (DVE single-partition reciprocal
  measured 4us(!) per call).
- Output stores via gpsimd SWDGE with fp16->f32 cast on the fly.
- Group epilogues (1/d, normalize, transpose-out) are DEFERRED into
  the next group's pipeline so the tensor engine never sits in the
  ln->exp->rdbc dependency bubble; ot and dps share one 2-buffered
  PSUM bank to make that legal within 8 banks.
"""

import numpy as np
from contextlib import ExitStack


def _ensure_ntff_hook():
    """The agent image's `antenv` lacks `axon_hooks`, so trn_boot's NTFF
    profile hook registration degrades silently and trace=True dies on
    import. Provide the missing module + register the ctypes hook."""
    import sys, types
    try:
        from antenv.axon_hooks import get_axon_ntff_profile_hook  # noqa
        return  # real module present
    except ImportError:
        pass
    try:
        import antenv
        mod = types.ModuleType("antenv.axon_hooks")
        _holder = [None]
        mod.set_axon_ntff_profile_hook = lambda h: _holder.__setitem__(0, h)
        mod.get_axon_ntff_profile_hook = lambda: _holder[0]
        sys.modules["antenv.axon_hooks"] = mod
        antenv.axon_hooks = mod
        from trn_agent_boot.trn_boot import _ntff_profile_via_ctypes
        mod.set_axon_ntff_profile_hook(
            _ntff_profile_via_ctypes("/opt/axon/libaxon_pjrt.so"))
    except Exception:
        pass


_ensure_ntff_hook()

import concourse.bass as bass
import concourse.tile as tile
from concourse import mybir
from concourse.bass_utils import run_bass_kernel_spmd
from concourse.masks import make_identity

F32 = mybir.dt.float32
F32R = mybir.dt.float32r
F16 = mybir.dt.float16

B, T, D, H = 16, 2048, 1024, 64
N_CORES = 8
BPC = B // N_CORES          # batches per core
P_DROP = 0.2
NB = T // 128               # 16 key chunks per batch
NG = T // 512               # 4 query groups per batch
GROUP = 4                   # key chunks per query group
MASK_NEG = -30000.0         # causal mask addend (fp16-representable)


# walrus here allows only ONE sync-wait command per instruction; Tile can
# attach several (e.g. its exit drain). Move extras onto same-engine NOPs.
def _split_excess_waits(nc):
    n = 0
    for f in nc.m.functions:
        for bb in f.blocks:
            new_insts = []
            changed = False
            for inst in bb.instructions:
                si = inst.sync_info
                if si is not None and si.on_wait and len(si.on_wait) > 1:
                    waits = list(si.on_wait)
                    extra, keep = waits[:-1], waits[-1:]
                    for i, w in enumerate(extra):
                        new_insts.append(mybir.InstNoOp(
                            name=f"{inst.name}-ws-{i}",
                            engine=inst.engine, ins=[], outs=[],
                            sync_info=mybir.SyncInfo(on_wait=[w], on_update=[]),
                            text_hint="waitsplit", bass_nofuse=True))
                        n += 1
                    si.on_wait = keep
                    changed = True
                new_insts.append(inst)
            if changed:
                bb.instructions[:] = new_insts
    return n


def _build(ctx: ExitStack, tc: "tile.TileContext", xt, wqk, wv, kfp, kdp, out):
    nc = tc.nc
    AF = mybir.ActivationFunctionType
    OP = mybir.AluOpType

    cpool = ctx.enter_context(tc.tile_pool(name="const", bufs=1))
    xpool = ctx.enter_context(tc.tile_pool(name="xt", bufs=2))
    qkvpool = ctx.enter_context(tc.tile_pool(name="qkv", bufs=2))
    vtpool = ctx.enter_context(tc.tile_pool(name="vt", bufs=2))
    kfpool = ctx.enter_context(tc.tile_pool(name="kf", bufs=2))
    kdpool = ctx.enter_context(tc.tile_pool(name="kd", bufs=4))
    eppool = ctx.enter_context(tc.tile_pool(name="ep", bufs=3))
    edpool = ctx.enter_context(tc.tile_pool(name="ed", bufs=3))
    pppool = ctx.enter_context(tc.tile_pool(name="pp", bufs=3))
    pdpool = ctx.enter_context(tc.tile_pool(name="pd", bufs=3))
    otsbpool = ctx.enter_context(tc.tile_pool(name="otsb", bufs=2))
    onsbpool = ctx.enter_context(tc.tile_pool(name="onsb", bufs=2))
    outpool = ctx.enter_context(tc.tile_pool(name="outsb", bufs=4))
    rdpool = ctx.enter_context(tc.tile_pool(name="rd", bufs=2))

    # PSUM: Sp 2x[128,1024] = 4 banks (pairs, v-proj, v-stage),
    # Sd 2x[128,512] = 2 banks (qk-proj, diag scores, rdbc),
    # combo (ot rows 0:64 + denom rows 64:128) 2x[128,512] -> 8 total.
    pspool = ctx.enter_context(tc.tile_pool(name="psp", bufs=2, space="PSUM"))
    psdool = ctx.enter_context(tc.tile_pool(name="psd", bufs=2, space="PSUM"))
    combops = ctx.enter_context(tc.tile_pool(name="combo", bufs=2, space="PSUM"))

    # deferred group epilogues: each entry is a closure that emits the
    # PE/DVE tail of a finished group; flushed between the next group's
    # producers so the tensor engine never idles in the 1/d dependency
    # chain.
    eptail = []

    def flush_eptail():
        while eptail:
            eptail.pop(0)()

    # ---- weights + first loads (emitted before const setup so the Pool
    # and SP rings start streaming immediately) --------------------------
    wqk_sb = cpool.tile([128, 8 * 128], F16)
    nc.sync.dma_start(wqk_sb[:], wqk)
    wv_sb = cpool.tile([128, 8 * H], F16)
    nc.sync.dma_start(wv_sb[:], wv)

    # staged x prefetch from the host-pre-tiled layout: each load is 128
    # contiguous 8KB descriptors (one per partition)
    xbigs = {}

    def load_x(b, quarter):
        if b >= BPC or quarter >= 4 or (b, quarter) in xbigs:
            return
        xb = xpool.tile([128, 8 * 512], F16, tag=f"x{b}{quarter}", bufs=1)
        if (b, quarter) == (0, 0):
            # split the critical first load so the projection can start
            # on the first half while the second is still in flight
            nc.gpsimd.dma_start(xb[:, 0:2048], xt[0, 0][:, 0:2048])
            nc.gpsimd.dma_start(xb[:, 2048:4096], xt[0, 0][:, 2048:4096])
        else:
            nc.gpsimd.dma_start(xb[:], xt[b, quarter])
        xbigs[(b, quarter)] = xb

    # diag keep-mask prefetch: ONE packed DMA per group (cols:
    # 512 | 384 | 256 | 128 for the four diagonal blocks)
    KD_OFF = [0, 512, 896, 1152]
    kds = {}

    def load_kd(b, g):
        if b >= BPC or g >= NG or (b, g) in kds:
            return
        kd = kdpool.tile([128, 1280], F16, tag="kd", bufs=3)
        nc.sync.dma_start(kd[:], kdp[b, g])
        kds[(b, g)] = kd

    load_x(0, 0)
    load_x(0, 1)
    load_kd(0, 0)

    # ---- constants ------------------------------------------------------
    ident_h = cpool.tile([128, 128], F16)
    make_identity(nc, ident_h[:])
    ones_w = cpool.tile([128, 64], F16)
    nc.gpsimd.memset(ones_w[:], 1.0)
    ones64_h = cpool.tile([1, 64], F16)
    nc.gpsimd.memset(ones64_h[:], 1.0)

    for b in range(BPC):
        # ---- phase A: projections ---------------------------------------
        # qkT[0:64,:] = q^T, qkT[64:128,:] = k^T ; v natural [s, H] tiles
        qkT = qkvpool.tile([128, T], F16, tag="qkT")
        # kT0 rows 0:63 = k^T; rows 64:127 stay ZERO so the scores
        # matmul can run with K=128 (measured faster than K=64) against
        # the full qkT as rhs -- the zero weight rows null out the k^T
        # half of qkT.
        kT0 = qkvpool.tile([128, T], F16, tag="kT0")
        nc.gpsimd.memset(kT0[64:128, :], 0.0)
        vT = vtpool.tile([64, T], F16, tag="vT")
        v_sb = qkvpool.tile([128, NB * H], F16, tag="v")

        for quarter in range(4):
            if quarter + 2 < 4:
                load_x(b, quarter + 2)
            else:
                load_x(b + 1, quarter + 2 - 4)
            col = 512 * quarter
            xb = xbigs.pop((b, quarter))
            ps = psdool.tile([128, 512], F32, tag="Sd")
            for c in range(8):
                nc.tensor.matmul(
                    ps[:], wqk_sb[:, 128 * c:128 * (c + 1)],
                    xb[:, 512 * c:512 * (c + 1)],
                    start=(c == 0), stop=(c == 7))
            nc.vector.tensor_copy(qkT[:, col:col + 512], ps[:])
            # matmul needs lhsT/rhs at the same base partition: move k^T
            # (psum rows 64..127) down to partitions 0..63 via DMA. On the
            # Act HWDGE ring so it never queues behind bulk keep loads.
            nc.scalar.dma_start(kT0[0:64, col:col + 512],
                                qkT[64:128, col:col + 512])
            ps2 = pspool.tile([64, 512], F32, tag="Sp",
                              padded_shape=[128, 1024])
            for c in range(8):
                nc.tensor.matmul(
                    ps2[:], wv_sb[:, H * c:H * (c + 1)],
                    xb[:, 512 * c:512 * (c + 1)],
                    start=(c == 0), stop=(c == 7))
            nc.scalar.copy(vT[:, col:col + 512], ps2[:])
            if quarter == 0:
                flush_eptail()  # prev batch's last-group tail
        qT = qkT
        kT = kT0

        # v: [64,T] -> natural [s, H] tiles, 8 PE transposes per PSUM bank
        for m in range(2):
            stage = pspool.tile([128, 512], F16, tag="Sp",
                                padded_shape=[128, 2048])
            for tloc in range(8):
                t = 8 * m + tloc
                nc.tensor.transpose(
                    stage[:, H * tloc:H * (tloc + 1)],
                    vT[:, 128 * t:128 * (t + 1)], ident_h[:64, :64])
            nc.vector.tensor_copy(
                v_sb[:, H * 8 * m:H * 8 * (m + 1)], stage[:])

        # ---- phase B: attention, per query group of 512 ------------------
        kfs = {}
        KF_OFF = {1: 0, 2: 2048, 3: 6144}

        def load_kf(g):
            if g > 3 or g == 0:
                return
            nf = 4 * g
            kf = kfpool.tile([128, nf * 512], F16, tag="kf",
                             padded_shape=[128, 12 * 512])
            nc.sync.dma_start(
                kf[:, :nf * 512],
                kfp[b, :, KF_OFF[g]:KF_OFF[g] + nf * 512])
            kfs[g] = kf

        load_kf(1)
        for g in range(NG):
            # small diag keep-masks FIRST so they never queue behind the
            # big kf transfer on the SP ring
            if g + 1 < NG:
                load_kd(b, g + 1)
            else:
                load_kd(b + 1, 0)
            if g >= 1:
                load_kf(g + 1)
            qcol = 512 * g
            kd_pack = kds.pop((b, g))
            kf = kfs.pop(g, None)
            combo = combops.tile([128, 512], F32, tag="combo")
            ot = combo[0:64, :]
            dps = combo[64:65, :]

            # work items: pairs of full chunks, then the 4 diagonal chunks
            items = [("pair", 2 * i) for i in range(2 * g)]
            items += [("diag", t) for t in range(4 * g, 4 * g + 4)]
            n_items = len(items)
            prod = {}

            def produce(i):
                kind, t = items[i]
                if kind == "pair":
                    sp = pspool.tile([128, 1024], F32, tag="Sp")
                    nc.tensor.matmul(
                        sp[:, 0:512], kT[:, 128 * t:128 * (t + 1)],
                        qT[:, qcol:qcol + 512], start=True, stop=True)
                    nc.tensor.matmul(
                        sp[:, 512:1024], kT[:, 128 * (t + 1):128 * (t + 2)],
                        qT[:, qcol:qcol + 512], start=True, stop=True)
                    E = eppool.tile([128, 1024], F16, tag="Ep")
                    nc.scalar.activation(
                        E[:], sp[:], AF.Exp, scale=float(H) ** -0.5)
                    prod[i] = E
                else:
                    qo = 128 * (t - 4 * g)
                    sd = psdool.tile([128, 512], F32, tag="Sd")
                    nc.tensor.matmul(
                        sd[:, qo:512], kT[:, 128 * t:128 * (t + 1)],
                        qT[:, qcol + qo:qcol + 512],
                        start=True, stop=True)
                    E = edpool.tile([128, 512], F16, tag="Ed")
                    nc.scalar.activation(
                        E[:, qo:512], sd[:, qo:512], AF.Exp,
                        scale=float(H) ** -0.5)
                    # causal mask: zero E above the diagonal of the
                    # 128x128 diagonal block (Pool; keeps PE out of it)
                    nc.gpsimd.affine_select(
                        out=E[:, qo:qo + 128], in_=E[:, qo:qo + 128],
                        compare_op=OP.is_ge, fill=0.0,
                        base=0, pattern=[[1, 128]], channel_multiplier=-1)
                    prod[i] = E

            def consume(i):
                kind, t = items[i]
                if kind == "pair":
                    E = prod.pop(i)
                    # denominator contributions (pre-dropout), written as
                    # 64 duplicate rows to partitions 64:128 of the shared
                    # ot/dps bank (M=1 matmuls measured ~110ns slower)
                    nc.tensor.matmul(
                        combo[64:128, :], ones_w[:], E[:, 0:512],
                        start=(i == 0), stop=False, skip_group_check=True)
                    nc.tensor.matmul(
                        combo[64:128, :], ones_w[:], E[:, 512:1024],
                        start=False, stop=False, skip_group_check=True)
                    Pp = pppool.tile([128, 1024], F16, tag="Pp")
                    nc.vector.tensor_mul(
                        Pp[:], kf[:, 512 * t:512 * (t + 2)], E[:])
                    nc.tensor.matmul(
                        ot[:], v_sb[:, H * t:H * (t + 1)], Pp[:, 0:512],
                        start=(i == 0), stop=False)
                    nc.tensor.matmul(
                        ot[:], v_sb[:, H * (t + 1):H * (t + 2)],
                        Pp[:, 512:1024],
                        start=False, stop=False)
                else:
                    qo = 128 * (t - 4 * g)
                    E = prod.pop(i)
                    nc.tensor.matmul(
                        combo[64:128, qo:512], ones_w[:], E[:, qo:512],
                        start=(i == 0), stop=(i == n_items - 1),
                        skip_group_check=True)
                    j = t - 4 * g
                    Pd = pdpool.tile([128, 512], F16, tag="Pd")
                    nc.vector.tensor_mul(
                        Pd[:, qo:512],
                        kd_pack[:, KD_OFF[j]:KD_OFF[j] + 512 - qo],
                        E[:, qo:512])
                    nc.tensor.matmul(
                        ot[:, qo:512], v_sb[:, H * t:H * (t + 1)],
                        Pd[:, qo:512],
                        start=(i == 0), stop=(i == n_items - 1))

            # software-pipelined: consumers trail producers by PD items and
            # are emitted in pairs, giving the tensor engine longer
            # back-to-back matmul runs. The previous group's epilogue tail
            # is flushed between the first producers so its PE ops land
            # when their inputs are long since ready.
            PD = 2
            pend = []
            for i in range(n_items):
                produce(i)
                if i in (1, 2) and eptail:
                    eptail.pop(0)()
                if i >= PD:
                    pend.append(i - PD)
                    if len(pend) == 2:
                        consume(pend[0])
                        consume(pend[1])
                        pend = []
            for i in pend:
                consume(i)
            for i in range(max(0, n_items - PD), n_items):
                consume(i)

            # ---- group epilogue -----------------------------------------
            # immediate part: free dps/ot quickly. 1/d as exp(-ln d) on
            # ScalarE (a [1,512] DVE reciprocal measured ~4us on HW).
            ot_sb = otsbpool.tile([64, 512], F32, tag="otsb")
            nc.vector.tensor_copy(ot_sb[:], combo[0:64, :])

            def make_tail(b, g, combo, ot_sb):
                def tail_pe():
                    ln_d = rdpool.tile([1, 512], F32, tag="rdf")
                    nc.scalar.activation(ln_d[:], combo[64:65, :], AF.Ln)
                    rd = rdpool.tile([1, 512], F16, tag="rd")
                    nc.scalar.activation(rd[:], ln_d[:], AF.Exp, scale=-1.0)
                    rdbc = psdool.tile([64, 512], F32, tag="Sd")
                    nc.tensor.matmul(rdbc[:], ones64_h[:], rd[:],
                                     start=True, stop=True)
                    on_sb = onsbpool.tile([64, 512], F16, tag="onsb")
                    nc.vector.tensor_mul(on_sb[:], ot_sb[:], rdbc[:])
                    tail_pe.on_sb = on_sb

                def tail_out():
                    on_sb = tail_pe.on_sb
                    stage = psdool.tile([128, 256], F16, tag="Sd",
                                        padded_shape=[128, 1024])
                    for cc in range(GROUP):
                        nc.tensor.transpose(
                            stage[:, 64 * cc:64 * (cc + 1)],
                            on_sb[:, 128 * cc:128 * (cc + 1)],
                            ident_h[:64, :64])
                    osb = outpool.tile([128, 256], F16, tag="osb")
                    nc.vector.tensor_copy(osb[:], stage[:])
                    # store with fp16->f32 cast via SWDGE
                    nc.gpsimd.dma_start(
                        out[b].rearrange("(c p) h -> p c h", p=128)
                           [:, GROUP * g:GROUP * (g + 1), :],
                        osb[:].rearrange("p (c h) -> p c h", c=GROUP))
                return [tail_pe, tail_out]

            eptail.extend(make_tail(b, g, combo, ot_sb))

    flush_eptail()


_CACHE = {}


def _get_nc():
    if "nc" not in _CACHE:
        nc = bass.Bass("TRN2", target_bir_lowering=False)
        # all inputs host-pre-tiled so every DMA is 128 contiguous
        # per-partition runs
        xt = nc.dram_tensor("xt", [BPC, 4, 128, 4096], F16,
                            kind="ExternalInput")
        wqk = nc.dram_tensor("wqk", [128, 8 * 128], F16, kind="ExternalInput")
        wv = nc.dram_tensor("wv", [128, 8 * H], F16, kind="ExternalInput")
        kfp = nc.dram_tensor("kfp", [BPC, 128, 12288], F16,
                             kind="ExternalInput")
        kdp = nc.dram_tensor("kdp", [BPC, 4, 128, 1280], F16,
                             kind="ExternalInput")
        out = nc.dram_tensor("out", [BPC, T, H], F32, kind="ExternalOutput")
        with tile.TileContext(nc) as tc:
            with ExitStack() as ctx:
                _build(ctx, tc, xt.ap(), wqk.ap(), wv.ap(), kfp.ap(),
                       kdp.ap(), out.ap())
        _split_excess_waits(nc)
        _CACHE["nc"] = nc
    return _CACHE["nc"]


def kernel(x, Wq, Wk, Wv, drop_u, _trace=False):
    x = np.asarray(x)
    drop_u = np.asarray(drop_u)

    nc = _get_nc()
    # x^T pre-tiled: xtp[b, q, p, 512c+tt] = x[b, 512q+tt, 128c+p]
    xt_full = x.astype(np.float16).transpose(0, 2, 1)      # [B, D, T]
    xtp = np.ascontiguousarray(
        xt_full.reshape(B, 8, 128, 4, 512).transpose(0, 3, 2, 1, 4)
        .reshape(B, 4, 128, 4096))
    keep = (drop_u >= np.float32(P_DROP))
    keep = (keep.astype(np.float16) * np.float16(1.0 / (1.0 - P_DROP)))
    keepT = keep.transpose(0, 2, 1)                        # [B, s, q]
    # full-chunk keep blocks, packed per group then per partition:
    # kfp[b, p, off_g + 512c + q] = keepT[b, 128c+p, 512g+q]
    kfp = np.empty((B, 128, 12288), np.float16)
    off = 0
    for g in (1, 2, 3):
        nf = 4 * g
        blk = keepT[:, 0:128 * nf, 512 * g:512 * (g + 1)]
        blk = (blk.reshape(B, nf, 128, 512).transpose(0, 2, 1, 3)
               .reshape(B, 128, nf * 512))
        kfp[:, :, off:off + nf * 512] = blk
        off += nf * 512
    # diagonal keep blocks packed per group: widths 512|384|256|128
    kdp = np.empty((B, 4, 128, 1280), np.float16)
    for g in range(4):
        off2 = 0
        for j in range(4):
            t = 4 * g + j
            qo = 128 * j
            w = 512 - qo
            kdp[:, g, :, off2:off2 + w] = \
                keepT[:, 128 * t:128 * (t + 1), 512 * g + qo:512 * (g + 1)]
            off2 += w
    # weights pre-tiled: w_sb[p, 128c+h] = W[128c+p, h]
    wqk0 = np.concatenate([np.asarray(Wq), np.asarray(Wk)],
                          axis=1).astype(np.float16)       # [D, 128]
    wqk = np.ascontiguousarray(
        wqk0.reshape(8, 128, 128).transpose(1, 0, 2).reshape(128, 1024))
    wv16 = np.ascontiguousarray(
        np.asarray(Wv).astype(np.float16)
        .reshape(8, 128, H).transpose(1, 0, 2).reshape(128, 8 * H))
    in_maps = []
    for c in range(N_CORES):
        lo = BPC * c
        in_maps.append({
            "xt": xtp[lo:lo + BPC],
            "wqk": wqk, "wv": wv16,
            "kfp": kfp[lo:lo + BPC],
            "kdp": kdp[lo:lo + BPC],
        })
    res = run_bass_kernel_spmd(
        nc, in_maps, core_ids=list(range(N_CORES)), trace=_trace)
    out = np.concatenate([res.results[c]["out"] for c in range(N_CORES)], axis=0)
    if _trace:
        kernel.last_exec_time_ns = res.exec_time_ns
        kernel.last_results = res
    return out
